# revision 9
# baseline (speedup 1.0000x reference)
"""Self-contained Trainium2 Bass kernel for GQA MultiHeadAttention with RoPE.

Problem: B=2, S=2048, D=1024, H=16 Q heads, KVH=4 KV heads, head_dim=64,
causal additive mask, f32.

Sharding: tensor-parallel over heads (TP=4: 4 Q heads + 1 KV head per shard)
x data-parallel over batch (DP=2) = 8 NeuronCores. Wo is sharded on its
input dim; the host sums the 4 partial outputs per batch element.

v3: ctx matmuls use probs as the stationary operand (out = [q, 65] per
k-tile) so the softmax rowsum is per-partition and normalization is one DVE
tensor_scalar; exp is the only ACT work. Q-projection and the output
projection are chopped into ~0.4us filler thunks drained one-per-attention-
unit so the PE's surplus work runs inside the ACT exp shadow.
"""

import os
import sys

for _p in ("/opt/trn_rl_repo", "/root/.axon_site/_ro/trn_rl_repo"):
    if os.path.isdir(_p) and _p not in sys.path:
        sys.path.insert(0, _p)

import numpy as np
import ml_dtypes

import concourse.bacc as bacc
import concourse.bass as bass
import concourse.tile as tile
from concourse import mybir
from concourse.bass_utils import run_bass_kernel_spmd

F32 = mybir.dt.float32
F32R = mybir.dt.float32r
BF16 = mybir.dt.bfloat16
AF = mybir.ActivationFunctionType

H, KVH, HD = 16, 4, 64
B, S, D = 2, 2048, 1024
TP = 4                      # head-parallel ways
SCALE = HD ** -0.5
NEG = -1e9
NT = S // 128               # 16 kv tiles
NQB = S // 512              # 4 q blocks


def _patch_act_tables():
    """Make Exp resolve only to natural_log_exp_and_others so the
    act-table-load pass emits one load instead of thrashing."""
    from concourse.hw_specs import get_activation_tables
    t = get_activation_tables("gen3")
    for name, fns in t.items():
        if name != "natural_log_exp_and_others":
            fns.discard(AF.Exp)
            fns.discard(AF.Ln)


def _build_nc(causal: bool):
    _patch_act_tables()
    nc = bacc.Bacc()

    hT = nc.declare_dram_parameter("hT", [D, S], BF16, isOutput=False)
    cs128 = nc.declare_dram_parameter("cs128", [128, S], BF16, isOutput=False)
    sn128 = nc.declare_dram_parameter("sn128", [128, S], BF16, isOutput=False)
    wq = nc.declare_dram_parameter("wq", [128, 8, 256], BF16, isOutput=False)
    wkv = nc.declare_dram_parameter("wkv", [128, 8, 128], BF16, isOutput=False)
    wo = nc.declare_dram_parameter("wo", [128, 2, D], BF16, isOutput=False)
    psigT = nc.declare_dram_parameter("psigT", [128, 128], F32R, isOutput=False)
    ident = nc.declare_dram_parameter("ident", [128, 128], F32R, isOutput=False)
    idb = nc.declare_dram_parameter("idb", [128, 128], BF16, isOutput=False)
    m01 = nc.declare_dram_parameter("m01", [128, 128], BF16, isOutput=False)
    outp = nc.declare_dram_parameter("out", [S, D], BF16, isOutput=True)

    with tile.TileContext(nc) as tc:
        with tc.tile_pool(name="hold", bufs=1) as hp:
            # load order matters: K/V projection inputs first so compute can
            # start ASAP; second hidden half + Wo and small consts later
            wkv_sb = hp.tile([128, 8, 128], BF16, name="wkv_sb", tag="wkv_sb")
            nc.sync.dma_start(out=wkv_sb, in_=wkv[:, :, :])
            ht_sb = [hp.tile([128, S], BF16, name=f"ht{c}", tag=f"ht{c}")
                     for c in range(8)]
            for c in range(8):
                nc.sync.dma_start(out=ht_sb[c][:, 0:1024],
                                  in_=hT[c * 128:(c + 1) * 128, 0:1024])
            psig_sb = hp.tile([128, 128], F32R, name="psig_sb", tag="psig_sb")
            nc.sync.dma_start(out=psig_sb, in_=psigT[:, :])
            cosf_sb = hp.tile([128, S], BF16, name="cosf_sb", tag="cosf_sb")
            sinf_sb = hp.tile([128, S], BF16, name="sinf_sb", tag="sinf_sb")
            nc.sync.dma_start(out=cosf_sb[:, 0:1024], in_=cs128[:, 0:1024])
            nc.sync.dma_start(out=sinf_sb[:, 0:1024], in_=sn128[:, 0:1024])
            id_sb = hp.tile([128, 128], F32R, name="id_sb", tag="id_sb")
            nc.sync.dma_start(out=id_sb, in_=ident[:, :])
            wq_sb = hp.tile([128, 8, 256], BF16, name="wq_sb", tag="wq_sb")
            nc.sync.dma_start(out=wq_sb, in_=wq[:, :, :])
            nc.sync.dma_start(out=cosf_sb[:, 1024:2048], in_=cs128[:, 1024:2048])
            nc.sync.dma_start(out=sinf_sb[:, 1024:2048], in_=sn128[:, 1024:2048])
            m01_sb = hp.tile([128, 128], BF16, name="m01_sb", tag="m01_sb")
            nc.sync.dma_start(out=m01_sb, in_=m01[:, :])
            for c in range(8):
                nc.sync.dma_start(out=ht_sb[c][:, 1024:2048],
                                  in_=hT[c * 128:(c + 1) * 128, 1024:2048])
            idb_sb = hp.tile([128, 128], BF16, name="idb_sb", tag="idb_sb")
            nc.sync.dma_start(out=idb_sb, in_=idb[:, :])
            wo_sb = hp.tile([128, 2, D], BF16, name="wo_sb", tag="wo_sb")
            nc.sync.dma_start(out=wo_sb, in_=wo[:, :, :])

            qTs = [hp.tile([128, S], BF16, name=f"qT{p}", tag=f"qT{p}")
                   for p in range(2)]
            kT = hp.tile([128, S], BF16, name="kTt", tag="kTt")
            vsm = hp.tile([128, NT, 65], BF16, name="vsm", tag="vsm")
            ctxTs = [[hp.tile([128, 512], BF16, name=f"ctxT{c}_{q}",
                              tag=f"ctxT{c}_{q}") for q in range(NQB)]
                     for c in range(2)]

            # ones column (65th) of vsm for the softmax denominator
            nc.vector.memset(vsm[:, :, 64:65], 1.0)

            with tc.tile_pool(name="psS", bufs=1, space="PSUM") as psS, \
                 tc.tile_pool(name="psC", bufs=1, space="PSUM") as psC, \
                 tc.tile_pool(name="psD", bufs=1, space="PSUM") as psD, \
                 tc.tile_pool(name="etp", bufs=1) as etp, \
                 tc.tile_pool(name="sbA", bufs=4) as sbA, \
                 tc.tile_pool(name="sbC", bufs=1) as sbC:

                # ---- filler queue: small PE-work thunks drained one per
                # attention unit so projections/output ride the exp shadow
                filler = []

                def drain(n=1):
                    for _ in range(n):
                        if filler:
                            filler.pop(0)[1]()

                def ensure(label):
                    rest, todo = [], []
                    for it in filler:
                        (todo if it[0] == label else rest).append(it)
                    filler[:] = rest
                    for _, th in todo:
                        th()

                def drain_all():
                    while filler:
                        filler.pop(0)[1]()

                # deferred phase-C emission (transposes + ctxT evicts)
                pending_c = []

                def flush_pending():
                    while pending_c:
                        pending_c.pop(0)()

                # ---------------- Phase A: projections + rope ----------------
                def queue_q_sc(pp, sc):
                    label = f"q{pp}sc{sc}"
                    csl = slice(512 * sc, 512 * sc + 512)
                    box = {}

                    def proj_a():
                        ps_q = psD.tile([128, 512], F32, name="ps_q",
                                        tag="ps_d", bufs=2)
                        for dc in range(4):
                            nc.tensor.matmul(
                                ps_q,
                                wq_sb[:, dc, 128 * pp:128 * pp + 128],
                                ht_sb[dc][:, csl],
                                start=(dc == 0), stop=False)
                        box["ps"] = ps_q

                    def proj_b():
                        ps_q = box["ps"]
                        for dc in range(4, 8):
                            nc.tensor.matmul(
                                ps_q,
                                wq_sb[:, dc, 128 * pp:128 * pp + 128],
                                ht_sb[dc][:, csl],
                                start=False, stop=(dc == 7))

                    def rope():
                        ps_q = box["ps"]
                        qraw = sbA.tile([128, 512], F32R, name="qraw",
                                        tag="qraw")
                        nc.scalar.copy(qraw, ps_q)
                        ps_rot = psD.tile([128, 512], F32, name="ps_rot",
                                          tag="ps_d", bufs=2)
                        nc.tensor.matmul(ps_rot, psig_sb.bitcast(F32R),
                                         qraw.bitcast(F32R),
                                         start=True, stop=True)
                        qc = sbA.tile([128, 512], F32, name="qc", tag="qc")
                        nc.gpsimd.tensor_mul(qc, qraw.bitcast(F32),
                                             cosf_sb[:, csl])
                        rtmp = sbA.tile([128, 512], F32, name="rtmp",
                                        tag="rtmp")
                        nc.vector.tensor_mul(rtmp, ps_rot, sinf_sb[:, csl])
                        nc.gpsimd.tensor_add(qTs[pp][:, csl], qc, rtmp)

                    filler.append((label, proj_a))
                    filler.append((label, proj_b))
                    filler.append((label, rope))

                def queue_kv_sc(sc, direct=False):
                    # K/V: kvT = [Wk|Wv].T @ h.T -> K rows 0:64, V rows 64:128
                    label = f"kvsc{sc}"
                    csl = slice(512 * sc, 512 * sc + 512)
                    box = {}

                    def proj_a():
                        ps_kv = psD.tile([128, 512], F32, name="ps_kv",
                                         tag="ps_d", bufs=2)
                        for dc in range(4):
                            nc.tensor.matmul(
                                ps_kv,
                                wkv_sb[:, dc, :],
                                ht_sb[dc][:, csl],
                                start=(dc == 0), stop=False)
                        box["ps"] = ps_kv

                    def proj_b():
                        ps_kv = box["ps"]
                        for dc in range(4, 8):
                            nc.tensor.matmul(
                                ps_kv,
                                wkv_sb[:, dc, :],
                                ht_sb[dc][:, csl],
                                start=False, stop=(dc == 7))

                    def krope():
                        kvraw = sbA.tile([128, 512], F32R, name="kvraw",
                                         tag="kvraw")
                        nc.scalar.copy(kvraw, box["ps"])
                        box["kvraw"] = kvraw
                        # rope on K rows
                        ps_krot = psD.tile([128, 512], F32, name="ps_krot",
                                           tag="ps_d", bufs=2)[0:64, :]
                        nc.tensor.matmul(ps_krot,
                                         psig_sb[0:64, 0:64].bitcast(F32R),
                                         kvraw[0:64, :].bitcast(F32R),
                                         start=True, stop=True)
                        kc = sbA.tile([64, 512], F32, name="kc", tag="kc")
                        nc.gpsimd.tensor_mul(kc, kvraw[0:64, :].bitcast(F32),
                                             cosf_sb[0:64, csl])
                        ktmp = sbA.tile([64, 512], F32, name="ktmp", tag="ktmp")
                        nc.vector.tensor_mul(ktmp, ps_krot, sinf_sb[0:64, csl])
                        nc.gpsimd.tensor_add(kT[0:64, csl], kc, ktmp)
                        # duplicate roped K to partitions 64:128 (engines
                        # cannot cross partitions; DMA can)
                        nc.sync.dma_start(out=kT[64:128, csl],
                                          in_=kT[0:64, csl])

                    def vt(pair):
                        # V: transpose 128-seq tiles into vsm (seq-major)
                        kvraw = box["kvraw"]
                        for tt in (2 * pair, 2 * pair + 1):
                            ti = 4 * sc + tt
                            ps_v = psD.tile([128, 512], F32, name="ps_v",
                                            tag="ps_d", bufs=2)[:, 0:64]
                            nc.tensor.matmul(
                                ps_v.bitcast(F32R),
                                kvraw[64:128, 128 * tt:128 * tt + 128].bitcast(F32R),
                                id_sb[64:128, 0:64].bitcast(F32R),
                                start=True, stop=True, is_transpose=True)
                            nc.vector.tensor_copy(vsm[:, ti, 0:64], ps_v)

                    thunks = [proj_a, proj_b, krope,
                              lambda: vt(0), lambda: vt(1)]
                    if direct:
                        for th in thunks:
                            th()
                    else:
                        for th in thunks:
                            filler.append((label, th))

                def queue_phase_d(dq):
                    label = f"pd{dq}"
                    for qt in range(4 * dq, 4 * dq + 4):
                        ct0 = ctxTs[0][qt // 4]
                        ct1 = ctxTs[1][qt // 4]
                        col = 128 * (qt % 4)
                        for nb in range(2):
                            def th(_qt=qt, _nb=nb, _ct0=ct0, _ct1=ct1,
                                   _col=col):
                                ps_o = psD.tile([128, 512], F32, name="ps_o",
                                                tag="ps_d", bufs=2)
                                nc.tensor.matmul(
                                    ps_o, _ct0[:, _col:_col + 128],
                                    wo_sb[:, 0, 512 * _nb:512 * _nb + 512],
                                    start=True, stop=False)
                                nc.tensor.matmul(
                                    ps_o, _ct1[:, _col:_col + 128],
                                    wo_sb[:, 1, 512 * _nb:512 * _nb + 512],
                                    start=False, stop=True)
                                ost = sbC.tile([128, 512], BF16, name="ost",
                                               tag="ost", bufs=6)
                                nc.vector.tensor_copy(ost, ps_o)
                                nc.sync.dma_start(
                                    out=outp[128 * _qt:128 * _qt + 128,
                                             512 * _nb:512 * _nb + 512],
                                    in_=ost)
                            filler.append((label, th))

                def build_head(qb, hh, sp):
                    h = 2 * sp + hh
                    off = 64 * (h % 2)
                    pp = h // 2
                    ps_ctx = psC.tile([128, 4, 65], F32, name=f"ps_ctx{hh}",
                                      tag="ps_ctx", bufs=2)
                    nfull = (4 * qb) if causal else NT
                    lastki = (4 * qb + 3) if causal else (NT - 1)
                    units = []

                    ctx_total = (16 * qb + 10) if causal else 64
                    ctx_cnt = [0]

                    def ctx_mms(et_ap, ki, jlist, base_idx=0):
                        # et_ap: probs chunk row; one [q,65] matmul per
                        # q-subtile j (chunk at base_idx+n within et_ap).
                        # All 4 q-subtile chains share one psum bank: the
                        # FIRST EMITTED matmul's start=True lazily zeroes
                        # the whole 2KB zero-region, every later matmul
                        # accumulates (disjoint addresses read as zero), and
                        # the LAST one closes the group with stop=True.
                        for idx, j in enumerate(jlist):
                            start = (ctx_cnt[0] == 0)
                            stop = (ctx_cnt[0] == ctx_total - 1)
                            ctx_cnt[0] += 1
                            o = 128 * (base_idx + idx)
                            nc.tensor.matmul(
                                ps_ctx[:, j, :],
                                et_ap[:, o:o + 128],
                                vsm[:, ki, 0:65],
                                start=start, stop=stop)

                    def mk_pair(kp):
                        box = {}

                        def s():
                            ps_s = psS.tile([128, 1024], F32, name="ps_s",
                                            tag="ps_s", bufs=2)
                            for jj in range(2):
                                ki = kp + jj
                                nc.tensor.matmul(
                                    ps_s[:, 512 * jj:512 * jj + 512],
                                    kT[off:off + 64,
                                       128 * ki:128 * ki + 128],
                                    qTs[pp][off:off + 64,
                                            512 * qb:512 * qb + 512],
                                    start=True, stop=True)
                            box["ps"] = ps_s

                        def ec():
                            et = etp.tile([128, 1024], BF16, name="et",
                                          tag="et", bufs=8)
                            nc.scalar.activation(et, box["ps"], AF.Exp,
                                                 scale=SCALE)
                            for jj in range(2):
                                ki = kp + jj
                                ctx_mms(et[:, 512 * jj:512 * jj + 512],
                                        ki, [0, 1, 2, 3])
                        return (s, ec)

                    def mk_diag(which):
                        # which=0: j=0 (span 512 @0) + j=1 (span 384 @512)
                        # which=1: j=2 (span 256 @0) + j=3 (span 128 @256)
                        box = {}
                        js = (0, 1) if which == 0 else (2, 3)
                        offs = (0, 512) if which == 0 else (0, 256)

                        def s():
                            ps_s = psS.tile([128, 1024], F32, name="ps_dg",
                                            tag="ps_s", bufs=2)
                            for j, o in zip(js, offs):
                                ki = 4 * qb + j
                                span = 512 - 128 * j
                                nc.tensor.matmul(
                                    ps_s[:, o:o + span],
                                    kT[off:off + 64,
                                       128 * ki:128 * ki + 128],
                                    qTs[pp][off:off + 64,
                                            512 * qb + 128 * j:
                                            512 * (qb + 1)],
                                    start=True, stop=True)
                            box["ps"] = ps_s

                        def ec():
                            wid = 896 if which == 0 else 384
                            et = etp.tile([128, 1024], BF16, name="etd",
                                          tag="et", bufs=8)
                            nc.scalar.activation(et[:, 0:wid],
                                                 box["ps"][:, 0:wid],
                                                 AF.Exp, scale=SCALE)
                            # mask the diagonal 128x128 chunk of each j
                            for j, o in zip(js, offs):
                                eng = nc.vector
                                eng.tensor_mul(et[:, o:o + 128],
                                               et[:, o:o + 128], m01_sb)
                            for j, o in zip(js, offs):
                                ki = 4 * qb + j
                                span = et[:, o:o + 512 - 128 * j]
                                jl = list(range(j, 4))
                                # unmasked q-subtiles first; the masked
                                # diagonal chunk (qt==j) last
                                ctx_mms(span, ki, jl[1:], base_idx=1)
                                ctx_mms(span, ki, jl[:1], base_idx=0)
                        return (s, ec)

                    if causal:
                        units.append(mk_diag(0))
                        units.append(mk_diag(1))
                    for kp in range(0, nfull, 2):
                        units.append(mk_pair(kp))

                    def phase_c(ctxns_h):
                        rc = sbC.tile([128, 4, 1], F32, name="rc", tag="rc",
                                      bufs=4)
                        nc.vector.reciprocal(rc, ps_ctx[:, :, 64:65])
                        for j in range(4):
                            cn = sbC.tile([128, 64], BF16, name="ctxn",
                                          tag="ctxn", bufs=16)
                            nc.vector.tensor_scalar_mul(
                                cn, ps_ctx[:, j, 0:64], rc[:, j, 0:1])
                            ctxns_h[j] = cn

                    return units, phase_c

                def emit_bc(qb, sp, post_flush=None, queue_fillers=None):
                    # attention for one (q block, head pair); the two heads'
                    # unit streams interleave so ACT always has a ready exp
                    ensure(f"q{sp}sc{qb}")
                    if qb > 0:
                        ensure(f"kvsc{qb}")
                    ctxns = [[None] * 4 for _ in range(2)]
                    u0, pc0 = build_head(qb, 0, sp)
                    u1, pc1 = build_head(qb, 1, sp)
                    n = len(u0)
                    u0[0][0]()
                    if post_flush is not None:
                        flush_pending()
                        post_flush()
                    if queue_fillers is not None:
                        queue_fillers()
                    u1[0][0]()
                    drain(1)
                    for i in range(n):
                        if i == 1:
                            # flush the previous block's transposes one
                            # round in, when their normalize chain is done
                            flush_pending()
                        if i + 1 < n:
                            u0[i + 1][0]()
                            drain(1)
                        u0[i][1]()
                        if i + 1 < n:
                            u1[i + 1][0]()
                            drain(1)
                        u1[i][1]()
                    pc0(ctxns[0])
                    pc1(ctxns[1])

                    def do_transposes(_sp=sp, _qb=qb, _ctxns=ctxns):
                        # 8 transposed chunks share one psum zero-region:
                        # first start=True zeroes it, the rest accumulate
                        # into disjoint (zeroed) addresses
                        ps_t = psD.tile([128, 512], F32, name="ps_t",
                                        tag="ps_d", bufs=2).bitcast(BF16)
                        for hh in range(2):
                            base = 64 * hh
                            for j in range(4):
                                nc.tensor.matmul(
                                    ps_t[base:base + 64,
                                         128 * j:128 * j + 128],
                                    _ctxns[hh][j], idb_sb[:, 0:128],
                                    start=(j == 0), stop=(j == 3),
                                    is_transpose=True)
                        for hh in range(2):
                            nc.vector.tensor_copy(
                                ctxTs[hh][_qb][64 * _sp:64 * _sp + 64, :],
                                ps_t[64 * hh:64 * hh + 64, 0:512])
                    pending_c.append(do_transposes)

                # ---- global emission order ----
                queue_kv_sc(0, direct=True)
                queue_q_sc(0, 0)
                ensure("q0sc0")
                queue_q_sc(1, 0)
                queue_kv_sc(1)
                queue_q_sc(0, 1)
                emit_bc(0, 0)
                queue_kv_sc(2)
                queue_q_sc(1, 1)
                emit_bc(0, 1)
                queue_q_sc(0, 2)
                queue_q_sc(1, 2)
                emit_bc(1, 0)
                queue_kv_sc(3)
                emit_bc(1, 1)

                def q20_fill():
                    queue_q_sc(0, 3)
                    queue_q_sc(1, 3)
                    queue_phase_d(0)
                emit_bc(2, 0, queue_fillers=q20_fill)
                emit_bc(2, 1, queue_fillers=lambda: queue_phase_d(1))
                emit_bc(3, 0, post_flush=lambda: queue_phase_d(2))
                emit_bc(3, 1)
                drain_all()
                flush_pending()
                # tail: last q block's output projection, double-width psum
                # slots from the now-idle attention ring for deep pipelining
                for qt in range(12, 16):
                    ps_o = psS.tile([128, 1024], F32, name="ps_ow",
                                    tag="ps_s", bufs=2)
                    col = 128 * (qt % 4)
                    for nb in range(2):
                        for c in range(2):
                            nc.tensor.matmul(
                                ps_o[:, 512 * nb:512 * nb + 512],
                                ctxTs[c][3][:, col:col + 128],
                                wo_sb[:, c, 512 * nb:512 * nb + 512],
                                start=(c == 0), stop=(c == 1))
                    ost = sbC.tile([128, 1024], BF16, name="ostw",
                                   tag="ostw", bufs=4)
                    nc.scalar.copy(ost[:, 0:512], ps_o[:, 0:512])
                    nc.vector.tensor_copy(ost[:, 512:1024], ps_o[:, 512:1024])
                    nc.sync.dma_start(
                        out=outp[128 * qt:128 * qt + 128, :], in_=ost)

    nc.compile()
    return nc


_NC_CACHE = {}


def _get_nc(causal: bool):
    if causal not in _NC_CACHE:
        _NC_CACHE[causal] = _build_nc(causal)
    return _NC_CACHE[causal]


def _host_consts():
    p = np.zeros((128, 128), np.float32)
    idx = np.arange(0, 128, 2)
    p[idx, idx + 1] = -1.0
    p[idx + 1, idx] = 1.0
    psigT = np.ascontiguousarray(p.T)
    ident = np.eye(128, dtype=np.float32)
    ident[64:128, 0:64] = np.eye(64, dtype=np.float32)
    idb = np.eye(128, dtype=ml_dtypes.bfloat16)
    m01 = (np.arange(128)[None, :] >= np.arange(128)[:, None])
    m01 = m01.astype(ml_dtypes.bfloat16)
    return psigT, ident, idb, m01


def _numpy_reference(hidden_states, cos, sin, attention_mask, Wq, Wk, Wv, Wo):
    """Generic-mask fallback, pure numpy port of the reference."""
    GROUPS = H // KVH

    def rope(x, c, s):
        c = c[:, None, :, :]
        s = s[:, None, :, :]
        x1, x2 = x[..., ::2], x[..., 1::2]
        xr = np.stack([x1 * c - x2 * s, x1 * s + x2 * c], axis=-1)
        return xr.reshape(x.shape)

    b, sq, d = hidden_states.shape
    q = (hidden_states @ Wq).reshape(b, sq, H, HD).transpose(0, 2, 1, 3)
    k = (hidden_states @ Wk).reshape(b, sq, KVH, HD).transpose(0, 2, 1, 3)
    v = (hidden_states @ Wv).reshape(b, sq, KVH, HD).transpose(0, 2, 1, 3)
    q = rope(q, cos, sin)
    k = rope(k, cos, sin)
    k = np.repeat(k, GROUPS, axis=1)
    v = np.repeat(v, GROUPS, axis=1)
    out = np.zeros((b, sq, d), np.float32)
    for bi in range(b):
        for hi in range(H):
            sc = (q[bi, hi] @ k[bi, hi].T) * SCALE + attention_mask[0, 0]
            sc = sc - sc.max(axis=-1, keepdims=True)
            e = np.exp(sc)
            pr = e / e.sum(axis=-1, keepdims=True)
            ctx = pr @ v[bi, hi]
            out[bi] += ctx @ Wo[hi * HD:(hi + 1) * HD]
    return out


def _make_in_maps(hs, cos, sin, Wq, Wk, Wv, Wo):
    psigT, ident, idb, m01 = _host_consts()
    chan_half = (np.arange(64) // 2)

    in_maps = []
    for core in range(8):
        b, t = core // TP, core % TP
        hT = np.ascontiguousarray(hs[b].T).astype(ml_dtypes.bfloat16)
        cs64v = np.ascontiguousarray(cos[b].T[chan_half, :])
        sn64v = np.ascontiguousarray(sin[b].T[chan_half, :])
        cs128v = np.ascontiguousarray(np.concatenate([cs64v, cs64v], axis=0)).astype(ml_dtypes.bfloat16)
        sn128v = np.ascontiguousarray(np.concatenate([sn64v, sn64v], axis=0)).astype(ml_dtypes.bfloat16)
        wq_s = Wq[:, t * 256:(t + 1) * 256].reshape(8, 128, 256)
        wq_s = np.ascontiguousarray(
            wq_s.transpose(1, 0, 2)).astype(ml_dtypes.bfloat16)
        wkv_s = np.concatenate([Wk[:, t * 64:(t + 1) * 64],
                                Wv[:, t * 64:(t + 1) * 64]],
                               axis=1).reshape(8, 128, 128)
        wkv_s = np.ascontiguousarray(
            wkv_s.transpose(1, 0, 2)).astype(ml_dtypes.bfloat16)
        wo_s = Wo[t * 256:(t + 1) * 256]
        # ctxT channel order per chunk: c0 = [h0|h2], c1 = [h1|h3]
        wo_p = np.concatenate([wo_s[0:64], wo_s[128:192],
                               wo_s[64:128], wo_s[192:256]],
                              axis=0).reshape(2, 128, 1024)
        wo_p = np.ascontiguousarray(
            wo_p.transpose(1, 0, 2)).astype(ml_dtypes.bfloat16)
        in_maps.append({
            "hT": hT, "cs128": cs128v, "sn128": sn128v,
            "wq": wq_s, "wkv": wkv_s, "wo": wo_p,
            "psigT": psigT, "ident": ident, "idb": idb, "m01": m01,
        })
    return in_maps


def kernel(**inputs) -> np.ndarray:
    hs = np.asarray(inputs["hidden_states"], np.float32)
    cos = np.asarray(inputs["cos"], np.float32)
    sin = np.asarray(inputs["sin"], np.float32)
    mask = np.asarray(inputs["attention_mask"], np.float32)
    Wq = np.asarray(inputs["Wq"], np.float32)
    Wk = np.asarray(inputs["Wk"], np.float32)
    Wv = np.asarray(inputs["Wv"], np.float32)
    Wo = np.asarray(inputs["Wo"], np.float32)

    m = mask.reshape(S, S)
    tril = np.tril(np.ones((S, S), dtype=bool))
    causal_ref = np.where(tril, np.float32(0.0), np.float32(NEG))
    if np.array_equal(m, causal_ref):
        causal = True
    elif not m.any():
        causal = False
    else:
        return _numpy_reference(hs, cos, sin, mask, Wq, Wk, Wv, Wo)

    nc = _get_nc(causal)
    in_maps = _make_in_maps(hs, cos, sin, Wq, Wk, Wv, Wo)
    res = run_bass_kernel_spmd(nc, in_maps, core_ids=list(range(8)))
    out = np.zeros((B, S, D), np.float32)
    for core in range(8):
        out[core // TP] += res.results[core]["out"].astype(np.float32)
    return out


# revision 10
# speedup vs baseline: 1.0018x; 1.0018x over previous
"""Self-contained Trainium2 Bass kernel for GQA MultiHeadAttention with RoPE.

Problem: B=2, S=2048, D=1024, H=16 Q heads, KVH=4 KV heads, head_dim=64,
causal additive mask, f32.

Sharding: tensor-parallel over heads (TP=4: 4 Q heads + 1 KV head per shard)
x data-parallel over batch (DP=2) = 8 NeuronCores. Wo is sharded on its
input dim; the host sums the 4 partial outputs per batch element.

Design notes (tuned against the TimelineSim cost model, HW-validated):
- ctx matmuls use probs as the STATIONARY operand (out = [128 qpos, 65]
  per k-tile, Ldweights is free) cutting ctx PE cost ~2.3x vs V-stationary,
  and putting the softmax rowsum per-partition: normalization is a single
  DVE reciprocal + tensor_scalar, no cross-partition reductions.
- exp is the only ACT-engine work; all PSUM evictions ride ACT (early,
  while idle) or DVE (GPSIMD cannot touch PSUM on real HW).
- the two heads of each (q-block, head-pair) interleave unit-by-unit so
  ACT always has a ready exp; K/V+Q projections and the output projection
  are chopped into ~0.4us filler thunks drained one-per-attention-unit,
  queued as late as dependencies allow so the late ACT-bound blocks stay
  fed; diag (masked) units run first within each block.
- all four ctx accumulation chains of a head share one 2KB PSUM bank:
  first-emitted matmul start=True zeroes the zero-region, the rest
  accumulate into disjoint addresses, last-emitted carries stop=True.
- bf16 everywhere precision allows (q/k/probs/V/ctxT/Wo/cos/sin/output
  partials); fp32 PSUM accumulation throughout keeps rel err ~4e-3.
"""

import os
import sys

for _p in ("/opt/trn_rl_repo", "/root/.axon_site/_ro/trn_rl_repo"):
    if os.path.isdir(_p) and _p not in sys.path:
        sys.path.insert(0, _p)

import numpy as np
import ml_dtypes

import concourse.bacc as bacc
import concourse.bass as bass
import concourse.tile as tile
from concourse import mybir
from concourse.bass_utils import run_bass_kernel_spmd

F32 = mybir.dt.float32
F32R = mybir.dt.float32r
BF16 = mybir.dt.bfloat16
AF = mybir.ActivationFunctionType

H, KVH, HD = 16, 4, 64
B, S, D = 2, 2048, 1024
TP = 4                      # head-parallel ways
SCALE = HD ** -0.5
NEG = -1e9
NT = S // 128               # 16 kv tiles
NQB = S // 512              # 4 q blocks


def _patch_act_tables():
    """Make Exp resolve only to natural_log_exp_and_others so the
    act-table-load pass emits one load instead of thrashing."""
    from concourse.hw_specs import get_activation_tables
    t = get_activation_tables("gen3")
    for name, fns in t.items():
        if name != "natural_log_exp_and_others":
            fns.discard(AF.Exp)
            fns.discard(AF.Ln)


def _build_nc(causal: bool):
    _patch_act_tables()
    nc = bacc.Bacc()

    hT = nc.declare_dram_parameter("hT", [D, S], BF16, isOutput=False)
    cs128 = nc.declare_dram_parameter("cs128", [128, S], BF16, isOutput=False)
    sn128 = nc.declare_dram_parameter("sn128", [128, S], BF16, isOutput=False)
    wq = nc.declare_dram_parameter("wq", [128, 8, 256], BF16, isOutput=False)
    wkv = nc.declare_dram_parameter("wkv", [128, 8, 128], BF16, isOutput=False)
    wo = nc.declare_dram_parameter("wo", [128, 2, D], BF16, isOutput=False)
    psigT = nc.declare_dram_parameter("psigT", [128, 128], F32R, isOutput=False)
    ident = nc.declare_dram_parameter("ident", [128, 128], F32R, isOutput=False)
    idb = nc.declare_dram_parameter("idb", [128, 128], BF16, isOutput=False)
    m01 = nc.declare_dram_parameter("m01", [128, 128], BF16, isOutput=False)
    outp = nc.declare_dram_parameter("out", [S, D], BF16, isOutput=True)

    with tile.TileContext(nc) as tc:
        with tc.tile_pool(name="hold", bufs=1) as hp:
            # load order matters: K/V projection inputs first so compute can
            # start ASAP; second hidden half + Wo and small consts later
            wkv_sb = hp.tile([128, 8, 128], BF16, name="wkv_sb", tag="wkv_sb")
            nc.sync.dma_start(out=wkv_sb, in_=wkv[:, :, :])
            ht_sb = [hp.tile([128, S], BF16, name=f"ht{c}", tag=f"ht{c}")
                     for c in range(8)]
            for c in range(8):
                nc.sync.dma_start(out=ht_sb[c][:, 0:1024],
                                  in_=hT[c * 128:(c + 1) * 128, 0:1024])
            psig_sb = hp.tile([128, 128], F32R, name="psig_sb", tag="psig_sb")
            nc.sync.dma_start(out=psig_sb, in_=psigT[:, :])
            cosf_sb = hp.tile([128, S], BF16, name="cosf_sb", tag="cosf_sb")
            sinf_sb = hp.tile([128, S], BF16, name="sinf_sb", tag="sinf_sb")
            nc.sync.dma_start(out=cosf_sb[:, 0:1024], in_=cs128[:, 0:1024])
            nc.sync.dma_start(out=sinf_sb[:, 0:1024], in_=sn128[:, 0:1024])
            id_sb = hp.tile([128, 128], F32R, name="id_sb", tag="id_sb")
            nc.sync.dma_start(out=id_sb, in_=ident[:, :])
            wq_sb = hp.tile([128, 8, 256], BF16, name="wq_sb", tag="wq_sb")
            nc.sync.dma_start(out=wq_sb, in_=wq[:, :, :])
            nc.sync.dma_start(out=cosf_sb[:, 1024:2048], in_=cs128[:, 1024:2048])
            nc.sync.dma_start(out=sinf_sb[:, 1024:2048], in_=sn128[:, 1024:2048])
            m01_sb = hp.tile([128, 128], BF16, name="m01_sb", tag="m01_sb")
            nc.sync.dma_start(out=m01_sb, in_=m01[:, :])
            for c in range(8):
                nc.sync.dma_start(out=ht_sb[c][:, 1024:2048],
                                  in_=hT[c * 128:(c + 1) * 128, 1024:2048])
            idb_sb = hp.tile([128, 128], BF16, name="idb_sb", tag="idb_sb")
            nc.sync.dma_start(out=idb_sb, in_=idb[:, :])
            wo_sb = hp.tile([128, 2, D], BF16, name="wo_sb", tag="wo_sb")
            nc.sync.dma_start(out=wo_sb, in_=wo[:, :, :])

            qTs = [hp.tile([128, S], BF16, name=f"qT{p}", tag=f"qT{p}")
                   for p in range(2)]
            kT = hp.tile([128, S], BF16, name="kTt", tag="kTt")
            vsm = hp.tile([128, NT, 65], BF16, name="vsm", tag="vsm")
            ctxTs = [[hp.tile([128, 512], BF16, name=f"ctxT{c}_{q}",
                              tag=f"ctxT{c}_{q}") for q in range(NQB)]
                     for c in range(2)]

            # ones column (65th) of vsm for the softmax denominator
            nc.vector.memset(vsm[:, :, 64:65], 1.0)

            with tc.tile_pool(name="psS", bufs=1, space="PSUM") as psS, \
                 tc.tile_pool(name="psC", bufs=1, space="PSUM") as psC, \
                 tc.tile_pool(name="psD", bufs=1, space="PSUM") as psD, \
                 tc.tile_pool(name="etp", bufs=1) as etp, \
                 tc.tile_pool(name="sbA", bufs=4) as sbA, \
                 tc.tile_pool(name="sbC", bufs=1) as sbC:

                # ---- filler queue: small PE-work thunks drained one per
                # attention unit so projections/output ride the exp shadow
                filler = []

                def drain(n=1):
                    for _ in range(n):
                        if filler:
                            filler.pop(0)[1]()

                def ensure(label):
                    rest, todo = [], []
                    for it in filler:
                        (todo if it[0] == label else rest).append(it)
                    filler[:] = rest
                    for _, th in todo:
                        th()

                def drain_all():
                    while filler:
                        filler.pop(0)[1]()

                # deferred phase-C emission (transposes + ctxT evicts)
                pending_c = []

                def flush_pending():
                    while pending_c:
                        pending_c.pop(0)()

                # ---------------- Phase A: projections + rope ----------------
                def queue_q_sc(pp, sc):
                    label = f"q{pp}sc{sc}"
                    csl = slice(512 * sc, 512 * sc + 512)
                    box = {}

                    def proj_a():
                        ps_q = psD.tile([128, 512], F32, name="ps_q",
                                        tag="ps_d", bufs=2)
                        for dc in range(4):
                            nc.tensor.matmul(
                                ps_q,
                                wq_sb[:, dc, 128 * pp:128 * pp + 128],
                                ht_sb[dc][:, csl],
                                start=(dc == 0), stop=False)
                        box["ps"] = ps_q

                    def proj_b():
                        ps_q = box["ps"]
                        for dc in range(4, 8):
                            nc.tensor.matmul(
                                ps_q,
                                wq_sb[:, dc, 128 * pp:128 * pp + 128],
                                ht_sb[dc][:, csl],
                                start=False, stop=(dc == 7))

                    def rope():
                        ps_q = box["ps"]
                        qraw = sbA.tile([128, 512], F32R, name="qraw",
                                        tag="qraw")
                        nc.scalar.copy(qraw, ps_q)
                        ps_rot = psD.tile([128, 512], F32, name="ps_rot",
                                          tag="ps_d", bufs=2)
                        nc.tensor.matmul(ps_rot, psig_sb.bitcast(F32R),
                                         qraw.bitcast(F32R),
                                         start=True, stop=True)
                        qc = sbA.tile([128, 512], F32, name="qc", tag="qc")
                        nc.gpsimd.tensor_mul(qc, qraw.bitcast(F32),
                                             cosf_sb[:, csl])
                        rtmp = sbA.tile([128, 512], F32, name="rtmp",
                                        tag="rtmp")
                        nc.vector.tensor_mul(rtmp, ps_rot, sinf_sb[:, csl])
                        nc.gpsimd.tensor_add(qTs[pp][:, csl], qc, rtmp)

                    filler.append((label, proj_a))
                    filler.append((label, proj_b))
                    filler.append((label, rope))

                def queue_kv_sc(sc, direct=False):
                    # K/V: kvT = [Wk|Wv].T @ h.T -> K rows 0:64, V rows 64:128
                    label = f"kvsc{sc}"
                    csl = slice(512 * sc, 512 * sc + 512)
                    box = {}

                    def proj_a():
                        ps_kv = psD.tile([128, 512], F32, name="ps_kv",
                                         tag="ps_d", bufs=2)
                        for dc in range(4):
                            nc.tensor.matmul(
                                ps_kv,
                                wkv_sb[:, dc, :],
                                ht_sb[dc][:, csl],
                                start=(dc == 0), stop=False)
                        box["ps"] = ps_kv

                    def proj_b():
                        ps_kv = box["ps"]
                        for dc in range(4, 8):
                            nc.tensor.matmul(
                                ps_kv,
                                wkv_sb[:, dc, :],
                                ht_sb[dc][:, csl],
                                start=False, stop=(dc == 7))

                    def krope():
                        kvraw = sbA.tile([128, 512], F32R, name="kvraw",
                                         tag="kvraw")
                        nc.scalar.copy(kvraw, box["ps"])
                        box["kvraw"] = kvraw
                        # rope on K rows
                        ps_krot = psD.tile([128, 512], F32, name="ps_krot",
                                           tag="ps_d", bufs=2)[0:64, :]
                        nc.tensor.matmul(ps_krot,
                                         psig_sb[0:64, 0:64].bitcast(F32R),
                                         kvraw[0:64, :].bitcast(F32R),
                                         start=True, stop=True)
                        kc = sbA.tile([64, 512], F32, name="kc", tag="kc")
                        nc.gpsimd.tensor_mul(kc, kvraw[0:64, :].bitcast(F32),
                                             cosf_sb[0:64, csl])
                        ktmp = sbA.tile([64, 512], F32, name="ktmp", tag="ktmp")
                        nc.vector.tensor_mul(ktmp, ps_krot, sinf_sb[0:64, csl])
                        nc.gpsimd.tensor_add(kT[0:64, csl], kc, ktmp)
                        # duplicate roped K to partitions 64:128 (engines
                        # cannot cross partitions; DMA can)
                        nc.sync.dma_start(out=kT[64:128, csl],
                                          in_=kT[0:64, csl])

                    def vt(pair):
                        # V: transpose 128-seq tiles into vsm (seq-major)
                        kvraw = box["kvraw"]
                        for tt in (2 * pair, 2 * pair + 1):
                            ti = 4 * sc + tt
                            ps_v = psD.tile([128, 512], F32, name="ps_v",
                                            tag="ps_d", bufs=2)[:, 0:64]
                            nc.tensor.matmul(
                                ps_v.bitcast(F32R),
                                kvraw[64:128, 128 * tt:128 * tt + 128].bitcast(F32R),
                                id_sb[64:128, 0:64].bitcast(F32R),
                                start=True, stop=True, is_transpose=True)
                            nc.vector.tensor_copy(vsm[:, ti, 0:64], ps_v)

                    thunks = [proj_a, proj_b, krope,
                              lambda: vt(0), lambda: vt(1)]
                    if direct:
                        for th in thunks:
                            th()
                    else:
                        for th in thunks:
                            filler.append((label, th))

                def queue_phase_d(dq):
                    label = f"pd{dq}"
                    for qt in range(4 * dq, 4 * dq + 4):
                        ct0 = ctxTs[0][qt // 4]
                        ct1 = ctxTs[1][qt // 4]
                        col = 128 * (qt % 4)
                        for nb in range(2):
                            def th(_qt=qt, _nb=nb, _ct0=ct0, _ct1=ct1,
                                   _col=col):
                                ps_o = psD.tile([128, 512], F32, name="ps_o",
                                                tag="ps_d", bufs=2)
                                nc.tensor.matmul(
                                    ps_o, _ct0[:, _col:_col + 128],
                                    wo_sb[:, 0, 512 * _nb:512 * _nb + 512],
                                    start=True, stop=False)
                                nc.tensor.matmul(
                                    ps_o, _ct1[:, _col:_col + 128],
                                    wo_sb[:, 1, 512 * _nb:512 * _nb + 512],
                                    start=False, stop=True)
                                ost = sbC.tile([128, 512], BF16, name="ost",
                                               tag="ost", bufs=6)
                                nc.vector.tensor_copy(ost, ps_o)
                                nc.sync.dma_start(
                                    out=outp[128 * _qt:128 * _qt + 128,
                                             512 * _nb:512 * _nb + 512],
                                    in_=ost)
                            filler.append((label, th))

                def build_head(qb, hh, sp):
                    h = 2 * sp + hh
                    off = 64 * (h % 2)
                    pp = h // 2
                    ps_ctx = psC.tile([128, 4, 65], F32, name=f"ps_ctx{hh}",
                                      tag="ps_ctx", bufs=2)
                    nfull = (4 * qb) if causal else NT
                    lastki = (4 * qb + 3) if causal else (NT - 1)
                    units = []

                    ctx_total = (16 * qb + 10) if causal else 64
                    ctx_cnt = [0]

                    def ctx_mms(et_ap, ki, jlist, base_idx=0):
                        # et_ap: probs chunk row; one [q,65] matmul per
                        # q-subtile j (chunk at base_idx+n within et_ap).
                        # All 4 q-subtile chains share one psum bank: the
                        # FIRST EMITTED matmul's start=True lazily zeroes
                        # the whole 2KB zero-region, every later matmul
                        # accumulates (disjoint addresses read as zero), and
                        # the LAST one closes the group with stop=True.
                        for idx, j in enumerate(jlist):
                            start = (ctx_cnt[0] == 0)
                            stop = (ctx_cnt[0] == ctx_total - 1)
                            ctx_cnt[0] += 1
                            o = 128 * (base_idx + idx)
                            nc.tensor.matmul(
                                ps_ctx[:, j, :],
                                et_ap[:, o:o + 128],
                                vsm[:, ki, 0:65],
                                start=start, stop=stop)

                    def mk_pair(kp):
                        box = {}

                        def s():
                            ps_s = psS.tile([128, 1024], F32, name="ps_s",
                                            tag="ps_s", bufs=2)
                            for jj in range(2):
                                ki = kp + jj
                                nc.tensor.matmul(
                                    ps_s[:, 512 * jj:512 * jj + 512],
                                    kT[off:off + 64,
                                       128 * ki:128 * ki + 128],
                                    qTs[pp][off:off + 64,
                                            512 * qb:512 * qb + 512],
                                    start=True, stop=True)
                            box["ps"] = ps_s

                        def ec():
                            et = etp.tile([128, 1024], BF16, name="et",
                                          tag="et", bufs=8)
                            nc.scalar.activation(et, box["ps"], AF.Exp,
                                                 scale=SCALE)
                            for jj in range(2):
                                ki = kp + jj
                                ctx_mms(et[:, 512 * jj:512 * jj + 512],
                                        ki, [0, 1, 2, 3])
                        return (s, ec)

                    def mk_diag(which):
                        # which=0: j=0 (span 512 @0) + j=1 (span 384 @512)
                        # which=1: j=2 (span 256 @0) + j=3 (span 128 @256)
                        box = {}
                        js = (0, 1) if which == 0 else (2, 3)
                        offs = (0, 512) if which == 0 else (0, 256)

                        def s():
                            ps_s = psS.tile([128, 1024], F32, name="ps_dg",
                                            tag="ps_s", bufs=2)
                            for j, o in zip(js, offs):
                                ki = 4 * qb + j
                                span = 512 - 128 * j
                                nc.tensor.matmul(
                                    ps_s[:, o:o + span],
                                    kT[off:off + 64,
                                       128 * ki:128 * ki + 128],
                                    qTs[pp][off:off + 64,
                                            512 * qb + 128 * j:
                                            512 * (qb + 1)],
                                    start=True, stop=True)
                            box["ps"] = ps_s

                        def ec():
                            wid = 896 if which == 0 else 384
                            et = etp.tile([128, 1024], BF16, name="etd",
                                          tag="et", bufs=8)
                            nc.scalar.activation(et[:, 0:wid],
                                                 box["ps"][:, 0:wid],
                                                 AF.Exp, scale=SCALE)
                            # mask the diagonal 128x128 chunk of each j
                            for j, o in zip(js, offs):
                                eng = nc.vector
                                eng.tensor_mul(et[:, o:o + 128],
                                               et[:, o:o + 128], m01_sb)
                            for j, o in zip(js, offs):
                                ki = 4 * qb + j
                                span = et[:, o:o + 512 - 128 * j]
                                jl = list(range(j, 4))
                                # unmasked q-subtiles first; the masked
                                # diagonal chunk (qt==j) last
                                ctx_mms(span, ki, jl[1:], base_idx=1)
                                ctx_mms(span, ki, jl[:1], base_idx=0)
                        return (s, ec)

                    if causal:
                        units.append(mk_diag(0))
                        units.append(mk_diag(1))
                    for kp in range(0, nfull, 2):
                        units.append(mk_pair(kp))

                    def phase_c(ctxns_h):
                        rc = sbC.tile([128, 4, 1], F32, name="rc", tag="rc",
                                      bufs=4)
                        nc.vector.reciprocal(rc, ps_ctx[:, :, 64:65])
                        for j in range(4):
                            cn = sbC.tile([128, 64], BF16, name="ctxn",
                                          tag="ctxn", bufs=16)
                            nc.vector.tensor_scalar_mul(
                                cn, ps_ctx[:, j, 0:64], rc[:, j, 0:1])
                            ctxns_h[j] = cn

                    return units, phase_c

                def emit_bc(qb, sp, post_flush=None, queue_fillers=None):
                    # attention for one (q block, head pair); the two heads'
                    # unit streams interleave so ACT always has a ready exp
                    ensure(f"q{sp}sc{qb}")
                    if qb > 0:
                        ensure(f"kvsc{qb}")
                    ctxns = [[None] * 4 for _ in range(2)]
                    u0, pc0 = build_head(qb, 0, sp)
                    u1, pc1 = build_head(qb, 1, sp)
                    n = len(u0)
                    u0[0][0]()
                    if post_flush is not None:
                        flush_pending()
                        post_flush()
                    if queue_fillers is not None:
                        queue_fillers()
                    u1[0][0]()
                    drain(1)
                    for i in range(n):
                        if i == 1:
                            # flush the previous block's transposes one
                            # round in, when their normalize chain is done
                            flush_pending()
                        if i + 1 < n:
                            u0[i + 1][0]()
                            drain(1)
                        u0[i][1]()
                        if i + 1 < n:
                            u1[i + 1][0]()
                            drain(1)
                        u1[i][1]()
                    pc0(ctxns[0])
                    pc1(ctxns[1])

                    def do_transposes(_sp=sp, _qb=qb, _ctxns=ctxns):
                        # 8 transposed chunks share one psum zero-region:
                        # first start=True zeroes it, the rest accumulate
                        # into disjoint (zeroed) addresses
                        ps_t = psD.tile([128, 512], F32, name="ps_t",
                                        tag="ps_d", bufs=2).bitcast(BF16)
                        for hh in range(2):
                            base = 64 * hh
                            for j in range(4):
                                nc.tensor.matmul(
                                    ps_t[base:base + 64,
                                         128 * j:128 * j + 128],
                                    _ctxns[hh][j], idb_sb[:, 0:128],
                                    start=(j == 0), stop=(j == 3),
                                    is_transpose=True)
                        for hh in range(2):
                            nc.vector.tensor_copy(
                                ctxTs[hh][_qb][64 * _sp:64 * _sp + 64, :],
                                ps_t[64 * hh:64 * hh + 64, 0:512])
                    pending_c.append(do_transposes)

                # ---- global emission order ----
                queue_kv_sc(0, direct=True)
                queue_q_sc(0, 0)
                ensure("q0sc0")
                queue_q_sc(1, 0)
                queue_kv_sc(1)
                queue_q_sc(0, 1)
                emit_bc(0, 0)
                queue_kv_sc(2)
                queue_q_sc(1, 1)
                emit_bc(0, 1)
                queue_q_sc(0, 2)
                queue_q_sc(1, 2)
                emit_bc(1, 0)
                queue_kv_sc(3)
                emit_bc(1, 1)

                def q20_fill():
                    queue_q_sc(0, 3)
                    queue_q_sc(1, 3)
                    queue_phase_d(0)
                emit_bc(2, 0, queue_fillers=q20_fill)
                emit_bc(2, 1, queue_fillers=lambda: queue_phase_d(1))
                emit_bc(3, 0, post_flush=lambda: queue_phase_d(2))
                emit_bc(3, 1)
                drain_all()
                flush_pending()
                # tail: last q block's output projection, double-width psum
                # slots from the now-idle attention ring for deep pipelining
                for qt in range(12, 16):
                    ps_o = psS.tile([128, 1024], F32, name="ps_ow",
                                    tag="ps_s", bufs=2)
                    col = 128 * (qt % 4)
                    for nb in range(2):
                        for c in range(2):
                            nc.tensor.matmul(
                                ps_o[:, 512 * nb:512 * nb + 512],
                                ctxTs[c][3][:, col:col + 128],
                                wo_sb[:, c, 512 * nb:512 * nb + 512],
                                start=(c == 0), stop=(c == 1))
                    ost = sbC.tile([128, 1024], BF16, name="ostw",
                                   tag="ostw", bufs=4)
                    nc.scalar.copy(ost[:, 0:512], ps_o[:, 0:512])
                    nc.vector.tensor_copy(ost[:, 512:1024], ps_o[:, 512:1024])
                    nc.sync.dma_start(
                        out=outp[128 * qt:128 * qt + 128, :], in_=ost)

    nc.compile()
    return nc


_NC_CACHE = {}


def _get_nc(causal: bool):
    if causal not in _NC_CACHE:
        _NC_CACHE[causal] = _build_nc(causal)
    return _NC_CACHE[causal]


def _host_consts():
    p = np.zeros((128, 128), np.float32)
    idx = np.arange(0, 128, 2)
    p[idx, idx + 1] = -1.0
    p[idx + 1, idx] = 1.0
    psigT = np.ascontiguousarray(p.T)
    ident = np.eye(128, dtype=np.float32)
    ident[64:128, 0:64] = np.eye(64, dtype=np.float32)
    idb = np.eye(128, dtype=ml_dtypes.bfloat16)
    m01 = (np.arange(128)[None, :] >= np.arange(128)[:, None])
    m01 = m01.astype(ml_dtypes.bfloat16)
    return psigT, ident, idb, m01


def _numpy_reference(hidden_states, cos, sin, attention_mask, Wq, Wk, Wv, Wo):
    """Generic-mask fallback, pure numpy port of the reference."""
    GROUPS = H // KVH

    def rope(x, c, s):
        c = c[:, None, :, :]
        s = s[:, None, :, :]
        x1, x2 = x[..., ::2], x[..., 1::2]
        xr = np.stack([x1 * c - x2 * s, x1 * s + x2 * c], axis=-1)
        return xr.reshape(x.shape)

    b, sq, d = hidden_states.shape
    q = (hidden_states @ Wq).reshape(b, sq, H, HD).transpose(0, 2, 1, 3)
    k = (hidden_states @ Wk).reshape(b, sq, KVH, HD).transpose(0, 2, 1, 3)
    v = (hidden_states @ Wv).reshape(b, sq, KVH, HD).transpose(0, 2, 1, 3)
    q = rope(q, cos, sin)
    k = rope(k, cos, sin)
    k = np.repeat(k, GROUPS, axis=1)
    v = np.repeat(v, GROUPS, axis=1)
    out = np.zeros((b, sq, d), np.float32)
    for bi in range(b):
        for hi in range(H):
            sc = (q[bi, hi] @ k[bi, hi].T) * SCALE + attention_mask[0, 0]
            sc = sc - sc.max(axis=-1, keepdims=True)
            e = np.exp(sc)
            pr = e / e.sum(axis=-1, keepdims=True)
            ctx = pr @ v[bi, hi]
            out[bi] += ctx @ Wo[hi * HD:(hi + 1) * HD]
    return out


def _make_in_maps(hs, cos, sin, Wq, Wk, Wv, Wo):
    psigT, ident, idb, m01 = _host_consts()
    chan_half = (np.arange(64) // 2)

    in_maps = []
    for core in range(8):
        b, t = core // TP, core % TP
        hT = np.ascontiguousarray(hs[b].T).astype(ml_dtypes.bfloat16)
        cs64v = np.ascontiguousarray(cos[b].T[chan_half, :])
        sn64v = np.ascontiguousarray(sin[b].T[chan_half, :])
        cs128v = np.ascontiguousarray(np.concatenate([cs64v, cs64v], axis=0)).astype(ml_dtypes.bfloat16)
        sn128v = np.ascontiguousarray(np.concatenate([sn64v, sn64v], axis=0)).astype(ml_dtypes.bfloat16)
        wq_s = Wq[:, t * 256:(t + 1) * 256].reshape(8, 128, 256)
        wq_s = np.ascontiguousarray(
            wq_s.transpose(1, 0, 2)).astype(ml_dtypes.bfloat16)
        wkv_s = np.concatenate([Wk[:, t * 64:(t + 1) * 64],
                                Wv[:, t * 64:(t + 1) * 64]],
                               axis=1).reshape(8, 128, 128)
        wkv_s = np.ascontiguousarray(
            wkv_s.transpose(1, 0, 2)).astype(ml_dtypes.bfloat16)
        wo_s = Wo[t * 256:(t + 1) * 256]
        # ctxT channel order per chunk: c0 = [h0|h2], c1 = [h1|h3]
        wo_p = np.concatenate([wo_s[0:64], wo_s[128:192],
                               wo_s[64:128], wo_s[192:256]],
                              axis=0).reshape(2, 128, 1024)
        wo_p = np.ascontiguousarray(
            wo_p.transpose(1, 0, 2)).astype(ml_dtypes.bfloat16)
        in_maps.append({
            "hT": hT, "cs128": cs128v, "sn128": sn128v,
            "wq": wq_s, "wkv": wkv_s, "wo": wo_p,
            "psigT": psigT, "ident": ident, "idb": idb, "m01": m01,
        })
    return in_maps


def kernel(**inputs) -> np.ndarray:
    hs = np.asarray(inputs["hidden_states"], np.float32)
    cos = np.asarray(inputs["cos"], np.float32)
    sin = np.asarray(inputs["sin"], np.float32)
    mask = np.asarray(inputs["attention_mask"], np.float32)
    Wq = np.asarray(inputs["Wq"], np.float32)
    Wk = np.asarray(inputs["Wk"], np.float32)
    Wv = np.asarray(inputs["Wv"], np.float32)
    Wo = np.asarray(inputs["Wo"], np.float32)

    m = mask.reshape(S, S)
    tril = np.tril(np.ones((S, S), dtype=bool))
    causal_ref = np.where(tril, np.float32(0.0), np.float32(NEG))
    if np.array_equal(m, causal_ref):
        causal = True
    elif not m.any():
        causal = False
    else:
        return _numpy_reference(hs, cos, sin, mask, Wq, Wk, Wv, Wo)

    nc = _get_nc(causal)
    in_maps = _make_in_maps(hs, cos, sin, Wq, Wk, Wv, Wo)
    res = run_bass_kernel_spmd(nc, in_maps, core_ids=list(range(8)))
    out = np.zeros((B, S, D), np.float32)
    for core in range(8):
        out[core // TP] += res.results[core]["out"].astype(np.float32)
    return out


# revision 11
# speedup vs baseline: 1.0087x; 1.0069x over previous
"""Self-contained Trainium2 Bass kernel for GQA MultiHeadAttention with RoPE.

Problem: B=2, S=2048, D=1024, H=16 Q heads, KVH=4 KV heads, head_dim=64,
causal additive mask, f32.

Sharding: tensor-parallel over heads (TP=4: 4 Q heads + 1 KV head per shard)
x data-parallel over batch (DP=2) = 8 NeuronCores. Wo is sharded on its
input dim; the host sums the 4 partial outputs per batch element.

Design notes (tuned against the TimelineSim cost model, HW-validated):
- ctx matmuls use probs as the STATIONARY operand (out = [128 qpos, 65]
  per k-tile, Ldweights is free) cutting ctx PE cost ~2.3x vs V-stationary,
  and putting the softmax rowsum per-partition: normalization is a single
  DVE reciprocal + tensor_scalar, no cross-partition reductions.
- exp is the only ACT-engine work; all PSUM evictions ride ACT (early,
  while idle) or DVE (GPSIMD cannot touch PSUM on real HW).
- the two heads of each (q-block, head-pair) interleave unit-by-unit so
  ACT always has a ready exp; K/V+Q projections and the output projection
  are chopped into ~0.4us filler thunks drained one-per-attention-unit,
  queued as late as dependencies allow so the late ACT-bound blocks stay
  fed; diag (masked) units run first within each block.
- all four ctx accumulation chains of a head share one 2KB PSUM bank:
  first-emitted matmul start=True zeroes the zero-region, the rest
  accumulate into disjoint addresses, last-emitted carries stop=True.
- bf16 everywhere precision allows (q/k/probs/V/ctxT/Wo/cos/sin/output
  partials); fp32 PSUM accumulation throughout keeps rel err ~4e-3.
"""

import os
import sys

for _p in ("/opt/trn_rl_repo", "/root/.axon_site/_ro/trn_rl_repo"):
    if os.path.isdir(_p) and _p not in sys.path:
        sys.path.insert(0, _p)

import numpy as np
import ml_dtypes

import concourse.bacc as bacc
import concourse.bass as bass
import concourse.tile as tile
from concourse import mybir
from concourse.bass_utils import run_bass_kernel_spmd

F32 = mybir.dt.float32
F32R = mybir.dt.float32r
BF16 = mybir.dt.bfloat16
AF = mybir.ActivationFunctionType

H, KVH, HD = 16, 4, 64
B, S, D = 2, 2048, 1024
TP = 4                      # head-parallel ways
SCALE = HD ** -0.5
NEG = -1e9
NT = S // 128               # 16 kv tiles
NQB = S // 512              # 4 q blocks


def _patch_act_tables():
    """Make Exp resolve only to natural_log_exp_and_others so the
    act-table-load pass emits one load instead of thrashing."""
    from concourse.hw_specs import get_activation_tables
    t = get_activation_tables("gen3")
    for name, fns in t.items():
        if name != "natural_log_exp_and_others":
            fns.discard(AF.Exp)
            fns.discard(AF.Ln)


def _build_nc(causal: bool):
    _patch_act_tables()
    nc = bacc.Bacc()

    hT = nc.declare_dram_parameter("hT", [D, S], BF16, isOutput=False)
    cs128 = nc.declare_dram_parameter("cs128", [128, S], BF16, isOutput=False)
    sn128 = nc.declare_dram_parameter("sn128", [128, S], BF16, isOutput=False)
    wq = nc.declare_dram_parameter("wq", [128, 8, 256], BF16, isOutput=False)
    wkv = nc.declare_dram_parameter("wkv", [128, 8, 128], BF16, isOutput=False)
    wo = nc.declare_dram_parameter("wo", [128, 2, D], BF16, isOutput=False)
    psigT = nc.declare_dram_parameter("psigT", [128, 128], F32R, isOutput=False)
    ident = nc.declare_dram_parameter("ident", [128, 128], F32R, isOutput=False)
    idb = nc.declare_dram_parameter("idb", [128, 128], BF16, isOutput=False)
    m01 = nc.declare_dram_parameter("m01", [128, 128], BF16, isOutput=False)
    outp = nc.declare_dram_parameter("out", [S, D], BF16, isOutput=True)

    with tile.TileContext(nc) as tc:
        with tc.tile_pool(name="hold", bufs=1) as hp:
            # load order matters: K/V projection inputs first so compute can
            # start ASAP; second hidden half + Wo and small consts later
            wkv_sb = hp.tile([128, 8, 128], BF16, name="wkv_sb", tag="wkv_sb")
            nc.sync.dma_start(out=wkv_sb, in_=wkv[:, :, :])
            ht_sb = [hp.tile([128, S], BF16, name=f"ht{c}", tag=f"ht{c}")
                     for c in range(8)]
            for c in range(8):
                eng = nc.sync if c % 2 == 0 else nc.gpsimd
                eng.dma_start(out=ht_sb[c][:, 0:1024],
                              in_=hT[c * 128:(c + 1) * 128, 0:1024])
            psig_sb = hp.tile([128, 128], F32R, name="psig_sb", tag="psig_sb")
            nc.sync.dma_start(out=psig_sb, in_=psigT[:, :])
            cosf_sb = hp.tile([128, S], BF16, name="cosf_sb", tag="cosf_sb")
            sinf_sb = hp.tile([128, S], BF16, name="sinf_sb", tag="sinf_sb")
            nc.gpsimd.dma_start(out=cosf_sb[:, 0:1024], in_=cs128[:, 0:1024])
            nc.sync.dma_start(out=sinf_sb[:, 0:1024], in_=sn128[:, 0:1024])
            id_sb = hp.tile([128, 128], F32R, name="id_sb", tag="id_sb")
            nc.sync.dma_start(out=id_sb, in_=ident[:, :])
            wq_sb = hp.tile([128, 8, 256], BF16, name="wq_sb", tag="wq_sb")
            nc.sync.dma_start(out=wq_sb, in_=wq[:, :, :])
            nc.sync.dma_start(out=cosf_sb[:, 1024:2048], in_=cs128[:, 1024:2048])
            nc.sync.dma_start(out=sinf_sb[:, 1024:2048], in_=sn128[:, 1024:2048])
            m01_sb = hp.tile([128, 128], BF16, name="m01_sb", tag="m01_sb")
            nc.sync.dma_start(out=m01_sb, in_=m01[:, :])
            for c in range(8):
                nc.sync.dma_start(out=ht_sb[c][:, 1024:2048],
                                  in_=hT[c * 128:(c + 1) * 128, 1024:2048])
            idb_sb = hp.tile([128, 128], BF16, name="idb_sb", tag="idb_sb")
            nc.sync.dma_start(out=idb_sb, in_=idb[:, :])
            wo_sb = hp.tile([128, 2, D], BF16, name="wo_sb", tag="wo_sb")
            nc.sync.dma_start(out=wo_sb, in_=wo[:, :, :])

            qTs = [hp.tile([128, S], BF16, name=f"qT{p}", tag=f"qT{p}")
                   for p in range(2)]
            kT = hp.tile([128, S], BF16, name="kTt", tag="kTt")
            vsm = hp.tile([128, NT, 65], BF16, name="vsm", tag="vsm")
            ctxTs = [[hp.tile([128, 512], BF16, name=f"ctxT{c}_{q}",
                              tag=f"ctxT{c}_{q}") for q in range(NQB)]
                     for c in range(2)]

            # ones column (65th) of vsm for the softmax denominator
            nc.vector.memset(vsm[:, :, 64:65], 1.0)

            with tc.tile_pool(name="psS", bufs=1, space="PSUM") as psS, \
                 tc.tile_pool(name="psC", bufs=1, space="PSUM") as psC, \
                 tc.tile_pool(name="psD", bufs=1, space="PSUM") as psD, \
                 tc.tile_pool(name="etp", bufs=1) as etp, \
                 tc.tile_pool(name="sbA", bufs=4) as sbA, \
                 tc.tile_pool(name="sbC", bufs=1) as sbC:

                # ---- filler queue: small PE-work thunks drained one per
                # attention unit so projections/output ride the exp shadow
                filler = []

                def drain(n=1):
                    for _ in range(n):
                        if filler:
                            filler.pop(0)[1]()

                def ensure(label):
                    rest, todo = [], []
                    for it in filler:
                        (todo if it[0] == label else rest).append(it)
                    filler[:] = rest
                    for _, th in todo:
                        th()

                def drain_all():
                    while filler:
                        filler.pop(0)[1]()

                # deferred phase-C emission (transposes + ctxT evicts)
                pending_c = []

                def flush_pending():
                    while pending_c:
                        pending_c.pop(0)()

                # ---------------- Phase A: projections + rope ----------------
                def queue_q_sc(pp, sc):
                    label = f"q{pp}sc{sc}"
                    csl = slice(512 * sc, 512 * sc + 512)
                    box = {}

                    def proj_a():
                        ps_q = psD.tile([128, 512], F32, name="ps_q",
                                        tag="ps_d", bufs=2)
                        for dc in range(4):
                            nc.tensor.matmul(
                                ps_q,
                                wq_sb[:, dc, 128 * pp:128 * pp + 128],
                                ht_sb[dc][:, csl],
                                start=(dc == 0), stop=False)
                        box["ps"] = ps_q

                    def proj_b():
                        ps_q = box["ps"]
                        for dc in range(4, 8):
                            nc.tensor.matmul(
                                ps_q,
                                wq_sb[:, dc, 128 * pp:128 * pp + 128],
                                ht_sb[dc][:, csl],
                                start=False, stop=(dc == 7))

                    def rope():
                        ps_q = box["ps"]
                        qraw = sbA.tile([128, 512], F32R, name="qraw",
                                        tag="qraw")
                        nc.scalar.copy(qraw, ps_q)
                        ps_rot = psD.tile([128, 512], F32, name="ps_rot",
                                          tag="ps_d", bufs=2)
                        nc.tensor.matmul(ps_rot, psig_sb.bitcast(F32R),
                                         qraw.bitcast(F32R),
                                         start=True, stop=True)
                        qc = sbA.tile([128, 512], F32, name="qc", tag="qc")
                        nc.gpsimd.tensor_mul(qc, qraw.bitcast(F32),
                                             cosf_sb[:, csl])
                        rtmp = sbA.tile([128, 512], F32, name="rtmp",
                                        tag="rtmp")
                        nc.vector.tensor_mul(rtmp, ps_rot, sinf_sb[:, csl])
                        nc.gpsimd.tensor_add(qTs[pp][:, csl], qc, rtmp)

                    filler.append((label, proj_a))
                    filler.append((label, proj_b))
                    filler.append((label, rope))

                def queue_kv_sc(sc, direct=False):
                    # K/V: kvT = [Wk|Wv].T @ h.T -> K rows 0:64, V rows 64:128
                    label = f"kvsc{sc}"
                    csl = slice(512 * sc, 512 * sc + 512)
                    box = {}

                    def proj_a():
                        ps_kv = psD.tile([128, 512], F32, name="ps_kv",
                                         tag="ps_d", bufs=2)
                        for dc in range(4):
                            nc.tensor.matmul(
                                ps_kv,
                                wkv_sb[:, dc, :],
                                ht_sb[dc][:, csl],
                                start=(dc == 0), stop=False)
                        box["ps"] = ps_kv

                    def proj_b():
                        ps_kv = box["ps"]
                        for dc in range(4, 8):
                            nc.tensor.matmul(
                                ps_kv,
                                wkv_sb[:, dc, :],
                                ht_sb[dc][:, csl],
                                start=False, stop=(dc == 7))

                    def krope():
                        kvraw = sbA.tile([128, 512], F32R, name="kvraw",
                                         tag="kvraw")
                        nc.scalar.copy(kvraw, box["ps"])
                        box["kvraw"] = kvraw
                        # rope on K rows
                        ps_krot = psD.tile([128, 512], F32, name="ps_krot",
                                           tag="ps_d", bufs=2)[0:64, :]
                        nc.tensor.matmul(ps_krot,
                                         psig_sb[0:64, 0:64].bitcast(F32R),
                                         kvraw[0:64, :].bitcast(F32R),
                                         start=True, stop=True)
                        kc = sbA.tile([64, 512], F32, name="kc", tag="kc")
                        nc.gpsimd.tensor_mul(kc, kvraw[0:64, :].bitcast(F32),
                                             cosf_sb[0:64, csl])
                        ktmp = sbA.tile([64, 512], F32, name="ktmp", tag="ktmp")
                        nc.vector.tensor_mul(ktmp, ps_krot, sinf_sb[0:64, csl])
                        nc.gpsimd.tensor_add(kT[0:64, csl], kc, ktmp)
                        # duplicate roped K to partitions 64:128 (engines
                        # cannot cross partitions; DMA can)
                        nc.sync.dma_start(out=kT[64:128, csl],
                                          in_=kT[0:64, csl])

                    def vt(pair):
                        # V: transpose 128-seq tiles into vsm (seq-major)
                        kvraw = box["kvraw"]
                        for tt in (2 * pair, 2 * pair + 1):
                            ti = 4 * sc + tt
                            ps_v = psD.tile([128, 512], F32, name="ps_v",
                                            tag="ps_d", bufs=2)[:, 0:64]
                            nc.tensor.matmul(
                                ps_v.bitcast(F32R),
                                kvraw[64:128, 128 * tt:128 * tt + 128].bitcast(F32R),
                                id_sb[64:128, 0:64].bitcast(F32R),
                                start=True, stop=True, is_transpose=True)
                            nc.vector.tensor_copy(vsm[:, ti, 0:64], ps_v)

                    thunks = [proj_a, proj_b, krope,
                              lambda: vt(0), lambda: vt(1)]
                    if direct:
                        for th in thunks:
                            th()
                    else:
                        for th in thunks:
                            filler.append((label, th))

                def queue_phase_d(dq):
                    label = f"pd{dq}"
                    for qt in range(4 * dq, 4 * dq + 4):
                        ct0 = ctxTs[0][qt // 4]
                        ct1 = ctxTs[1][qt // 4]
                        col = 128 * (qt % 4)
                        for nb in range(2):
                            def th(_qt=qt, _nb=nb, _ct0=ct0, _ct1=ct1,
                                   _col=col):
                                ps_o = psD.tile([128, 512], F32, name="ps_o",
                                                tag="ps_d", bufs=2)
                                nc.tensor.matmul(
                                    ps_o, _ct0[:, _col:_col + 128],
                                    wo_sb[:, 0, 512 * _nb:512 * _nb + 512],
                                    start=True, stop=False)
                                nc.tensor.matmul(
                                    ps_o, _ct1[:, _col:_col + 128],
                                    wo_sb[:, 1, 512 * _nb:512 * _nb + 512],
                                    start=False, stop=True)
                                ost = sbC.tile([128, 512], BF16, name="ost",
                                               tag="ost", bufs=6)
                                nc.vector.tensor_copy(ost, ps_o)
                                nc.sync.dma_start(
                                    out=outp[128 * _qt:128 * _qt + 128,
                                             512 * _nb:512 * _nb + 512],
                                    in_=ost)
                            filler.append((label, th))

                def build_head(qb, hh, sp):
                    h = 2 * sp + hh
                    off = 64 * (h % 2)
                    pp = h // 2
                    ps_ctx = psC.tile([128, 4, 65], F32, name=f"ps_ctx{hh}",
                                      tag="ps_ctx", bufs=2)
                    nfull = (4 * qb) if causal else NT
                    lastki = (4 * qb + 3) if causal else (NT - 1)
                    units = []

                    ctx_total = (16 * qb + 10) if causal else 64
                    ctx_cnt = [0]

                    def ctx_mms(et_ap, ki, jlist, base_idx=0):
                        # et_ap: probs chunk row; one [q,65] matmul per
                        # q-subtile j (chunk at base_idx+n within et_ap).
                        # All 4 q-subtile chains share one psum bank: the
                        # FIRST EMITTED matmul's start=True lazily zeroes
                        # the whole 2KB zero-region, every later matmul
                        # accumulates (disjoint addresses read as zero), and
                        # the LAST one closes the group with stop=True.
                        for idx, j in enumerate(jlist):
                            start = (ctx_cnt[0] == 0)
                            stop = (ctx_cnt[0] == ctx_total - 1)
                            ctx_cnt[0] += 1
                            o = 128 * (base_idx + idx)
                            nc.tensor.matmul(
                                ps_ctx[:, j, :],
                                et_ap[:, o:o + 128],
                                vsm[:, ki, 0:65],
                                start=start, stop=stop)

                    def mk_pair(kp):
                        box = {}

                        def s():
                            ps_s = psS.tile([128, 1024], F32, name="ps_s",
                                            tag="ps_s", bufs=2)
                            for jj in range(2):
                                ki = kp + jj
                                nc.tensor.matmul(
                                    ps_s[:, 512 * jj:512 * jj + 512],
                                    kT[off:off + 64,
                                       128 * ki:128 * ki + 128],
                                    qTs[pp][off:off + 64,
                                            512 * qb:512 * qb + 512],
                                    start=True, stop=True)
                            box["ps"] = ps_s

                        def ec():
                            et = etp.tile([128, 1024], BF16, name="et",
                                          tag="et", bufs=8)
                            nc.scalar.activation(et, box["ps"], AF.Exp,
                                                 scale=SCALE)
                            for jj in range(2):
                                ki = kp + jj
                                ctx_mms(et[:, 512 * jj:512 * jj + 512],
                                        ki, [0, 1, 2, 3])
                        return (s, ec)

                    def mk_diag(which):
                        # which=0: j=0 (span 512 @0) + j=1 (span 384 @512)
                        # which=1: j=2 (span 256 @0) + j=3 (span 128 @256)
                        box = {}
                        js = (0, 1) if which == 0 else (2, 3)
                        offs = (0, 512) if which == 0 else (0, 256)

                        def s():
                            ps_s = psS.tile([128, 1024], F32, name="ps_dg",
                                            tag="ps_s", bufs=2)
                            for j, o in zip(js, offs):
                                ki = 4 * qb + j
                                span = 512 - 128 * j
                                nc.tensor.matmul(
                                    ps_s[:, o:o + span],
                                    kT[off:off + 64,
                                       128 * ki:128 * ki + 128],
                                    qTs[pp][off:off + 64,
                                            512 * qb + 128 * j:
                                            512 * (qb + 1)],
                                    start=True, stop=True)
                            box["ps"] = ps_s

                        def ec():
                            wid = 896 if which == 0 else 384
                            et = etp.tile([128, 1024], BF16, name="etd",
                                          tag="et", bufs=8)
                            nc.scalar.activation(et[:, 0:wid],
                                                 box["ps"][:, 0:wid],
                                                 AF.Exp, scale=SCALE)
                            # mask the diagonal 128x128 chunk of each j
                            for j, o in zip(js, offs):
                                eng = nc.vector
                                eng.tensor_mul(et[:, o:o + 128],
                                               et[:, o:o + 128], m01_sb)
                            for j, o in zip(js, offs):
                                ki = 4 * qb + j
                                span = et[:, o:o + 512 - 128 * j]
                                jl = list(range(j, 4))
                                # unmasked q-subtiles first; the masked
                                # diagonal chunk (qt==j) last
                                ctx_mms(span, ki, jl[1:], base_idx=1)
                                ctx_mms(span, ki, jl[:1], base_idx=0)
                        return (s, ec)

                    if causal:
                        units.append(mk_diag(0))
                        units.append(mk_diag(1))
                    for kp in range(0, nfull, 2):
                        units.append(mk_pair(kp))

                    def phase_c(ctxns_h):
                        rc = sbC.tile([128, 4, 1], F32, name="rc", tag="rc",
                                      bufs=4)
                        nc.vector.reciprocal(rc, ps_ctx[:, :, 64:65])
                        for j in range(4):
                            cn = sbC.tile([128, 64], BF16, name="ctxn",
                                          tag="ctxn", bufs=16)
                            nc.vector.tensor_scalar_mul(
                                cn, ps_ctx[:, j, 0:64], rc[:, j, 0:1])
                            ctxns_h[j] = cn

                    return units, phase_c

                def emit_bc(qb, sp, post_flush=None, queue_fillers=None):
                    # attention for one (q block, head pair); the two heads'
                    # unit streams interleave so ACT always has a ready exp
                    ensure(f"q{sp}sc{qb}")
                    if qb > 0:
                        ensure(f"kvsc{qb}")
                    ctxns = [[None] * 4 for _ in range(2)]
                    u0, pc0 = build_head(qb, 0, sp)
                    u1, pc1 = build_head(qb, 1, sp)
                    n = len(u0)
                    u0[0][0]()
                    if post_flush is not None:
                        flush_pending()
                        post_flush()
                    if queue_fillers is not None:
                        queue_fillers()
                    u1[0][0]()
                    drain(1)
                    for i in range(n):
                        if i == 1:
                            # flush the previous block's transposes one
                            # round in, when their normalize chain is done
                            flush_pending()
                        if i + 1 < n:
                            u0[i + 1][0]()
                            drain(1)
                        u0[i][1]()
                        if i + 1 < n:
                            u1[i + 1][0]()
                            drain(1)
                        u1[i][1]()
                    pc0(ctxns[0])
                    pc1(ctxns[1])

                    def do_transposes(_sp=sp, _qb=qb, _ctxns=ctxns):
                        # 8 transposed chunks share one psum zero-region:
                        # first start=True zeroes it, the rest accumulate
                        # into disjoint (zeroed) addresses
                        ps_t = psD.tile([128, 512], F32, name="ps_t",
                                        tag="ps_d", bufs=2).bitcast(BF16)
                        for hh in range(2):
                            base = 64 * hh
                            for j in range(4):
                                nc.tensor.matmul(
                                    ps_t[base:base + 64,
                                         128 * j:128 * j + 128],
                                    _ctxns[hh][j], idb_sb[:, 0:128],
                                    start=(j == 0), stop=(j == 3),
                                    is_transpose=True)
                        for hh in range(2):
                            nc.vector.tensor_copy(
                                ctxTs[hh][_qb][64 * _sp:64 * _sp + 64, :],
                                ps_t[64 * hh:64 * hh + 64, 0:512])
                    pending_c.append(do_transposes)

                # ---- global emission order ----
                queue_kv_sc(0, direct=True)
                queue_q_sc(0, 0)
                ensure("q0sc0")
                queue_q_sc(1, 0)
                queue_kv_sc(1)
                queue_q_sc(0, 1)
                emit_bc(0, 0)
                queue_kv_sc(2)
                queue_q_sc(1, 1)
                emit_bc(0, 1)
                queue_q_sc(0, 2)
                queue_q_sc(1, 2)
                emit_bc(1, 0)
                queue_kv_sc(3)
                emit_bc(1, 1)

                def q20_fill():
                    queue_q_sc(0, 3)
                    queue_q_sc(1, 3)
                    queue_phase_d(0)
                emit_bc(2, 0, queue_fillers=q20_fill)
                emit_bc(2, 1, queue_fillers=lambda: queue_phase_d(1))
                emit_bc(3, 0, post_flush=lambda: queue_phase_d(2))
                emit_bc(3, 1)
                drain_all()
                flush_pending()
                # tail: last q block's output projection, double-width psum
                # slots from the now-idle attention ring for deep pipelining
                for qt in range(12, 16):
                    ps_o = psS.tile([128, 1024], F32, name="ps_ow",
                                    tag="ps_s", bufs=2)
                    col = 128 * (qt % 4)
                    for nb in range(2):
                        for c in range(2):
                            nc.tensor.matmul(
                                ps_o[:, 512 * nb:512 * nb + 512],
                                ctxTs[c][3][:, col:col + 128],
                                wo_sb[:, c, 512 * nb:512 * nb + 512],
                                start=(c == 0), stop=(c == 1))
                    ost = sbC.tile([128, 1024], BF16, name="ostw",
                                   tag="ostw", bufs=4)
                    nc.scalar.copy(ost[:, 0:512], ps_o[:, 0:512])
                    nc.vector.tensor_copy(ost[:, 512:1024], ps_o[:, 512:1024])
                    nc.sync.dma_start(
                        out=outp[128 * qt:128 * qt + 128, :], in_=ost)

    nc.compile()
    return nc


_NC_CACHE = {}


def _get_nc(causal: bool):
    if causal not in _NC_CACHE:
        _NC_CACHE[causal] = _build_nc(causal)
    return _NC_CACHE[causal]


def _host_consts():
    p = np.zeros((128, 128), np.float32)
    idx = np.arange(0, 128, 2)
    p[idx, idx + 1] = -1.0
    p[idx + 1, idx] = 1.0
    psigT = np.ascontiguousarray(p.T)
    ident = np.eye(128, dtype=np.float32)
    ident[64:128, 0:64] = np.eye(64, dtype=np.float32)
    idb = np.eye(128, dtype=ml_dtypes.bfloat16)
    m01 = (np.arange(128)[None, :] >= np.arange(128)[:, None])
    m01 = m01.astype(ml_dtypes.bfloat16)
    return psigT, ident, idb, m01


def _numpy_reference(hidden_states, cos, sin, attention_mask, Wq, Wk, Wv, Wo):
    """Generic-mask fallback, pure numpy port of the reference."""
    GROUPS = H // KVH

    def rope(x, c, s):
        c = c[:, None, :, :]
        s = s[:, None, :, :]
        x1, x2 = x[..., ::2], x[..., 1::2]
        xr = np.stack([x1 * c - x2 * s, x1 * s + x2 * c], axis=-1)
        return xr.reshape(x.shape)

    b, sq, d = hidden_states.shape
    q = (hidden_states @ Wq).reshape(b, sq, H, HD).transpose(0, 2, 1, 3)
    k = (hidden_states @ Wk).reshape(b, sq, KVH, HD).transpose(0, 2, 1, 3)
    v = (hidden_states @ Wv).reshape(b, sq, KVH, HD).transpose(0, 2, 1, 3)
    q = rope(q, cos, sin)
    k = rope(k, cos, sin)
    k = np.repeat(k, GROUPS, axis=1)
    v = np.repeat(v, GROUPS, axis=1)
    out = np.zeros((b, sq, d), np.float32)
    for bi in range(b):
        for hi in range(H):
            sc = (q[bi, hi] @ k[bi, hi].T) * SCALE + attention_mask[0, 0]
            sc = sc - sc.max(axis=-1, keepdims=True)
            e = np.exp(sc)
            pr = e / e.sum(axis=-1, keepdims=True)
            ctx = pr @ v[bi, hi]
            out[bi] += ctx @ Wo[hi * HD:(hi + 1) * HD]
    return out


def _make_in_maps(hs, cos, sin, Wq, Wk, Wv, Wo):
    psigT, ident, idb, m01 = _host_consts()
    chan_half = (np.arange(64) // 2)

    in_maps = []
    for core in range(8):
        b, t = core // TP, core % TP
        hT = np.ascontiguousarray(hs[b].T).astype(ml_dtypes.bfloat16)
        cs64v = np.ascontiguousarray(cos[b].T[chan_half, :])
        sn64v = np.ascontiguousarray(sin[b].T[chan_half, :])
        cs128v = np.ascontiguousarray(np.concatenate([cs64v, cs64v], axis=0)).astype(ml_dtypes.bfloat16)
        sn128v = np.ascontiguousarray(np.concatenate([sn64v, sn64v], axis=0)).astype(ml_dtypes.bfloat16)
        wq_s = Wq[:, t * 256:(t + 1) * 256].reshape(8, 128, 256)
        wq_s = np.ascontiguousarray(
            wq_s.transpose(1, 0, 2)).astype(ml_dtypes.bfloat16)
        wkv_s = np.concatenate([Wk[:, t * 64:(t + 1) * 64],
                                Wv[:, t * 64:(t + 1) * 64]],
                               axis=1).reshape(8, 128, 128)
        wkv_s = np.ascontiguousarray(
            wkv_s.transpose(1, 0, 2)).astype(ml_dtypes.bfloat16)
        wo_s = Wo[t * 256:(t + 1) * 256]
        # ctxT channel order per chunk: c0 = [h0|h2], c1 = [h1|h3]
        wo_p = np.concatenate([wo_s[0:64], wo_s[128:192],
                               wo_s[64:128], wo_s[192:256]],
                              axis=0).reshape(2, 128, 1024)
        wo_p = np.ascontiguousarray(
            wo_p.transpose(1, 0, 2)).astype(ml_dtypes.bfloat16)
        in_maps.append({
            "hT": hT, "cs128": cs128v, "sn128": sn128v,
            "wq": wq_s, "wkv": wkv_s, "wo": wo_p,
            "psigT": psigT, "ident": ident, "idb": idb, "m01": m01,
        })
    return in_maps


def kernel(**inputs) -> np.ndarray:
    hs = np.asarray(inputs["hidden_states"], np.float32)
    cos = np.asarray(inputs["cos"], np.float32)
    sin = np.asarray(inputs["sin"], np.float32)
    mask = np.asarray(inputs["attention_mask"], np.float32)
    Wq = np.asarray(inputs["Wq"], np.float32)
    Wk = np.asarray(inputs["Wk"], np.float32)
    Wv = np.asarray(inputs["Wv"], np.float32)
    Wo = np.asarray(inputs["Wo"], np.float32)

    m = mask.reshape(S, S)
    tril = np.tril(np.ones((S, S), dtype=bool))
    causal_ref = np.where(tril, np.float32(0.0), np.float32(NEG))
    if np.array_equal(m, causal_ref):
        causal = True
    elif not m.any():
        causal = False
    else:
        return _numpy_reference(hs, cos, sin, mask, Wq, Wk, Wv, Wo)

    nc = _get_nc(causal)
    in_maps = _make_in_maps(hs, cos, sin, Wq, Wk, Wv, Wo)
    res = run_bass_kernel_spmd(nc, in_maps, core_ids=list(range(8)))
    out = np.zeros((B, S, D), np.float32)
    for core in range(8):
        out[core // TP] += res.results[core]["out"].astype(np.float32)
    return out


# revision 12
# speedup vs baseline: 1.0185x; 1.0097x over previous
"""Self-contained Trainium2 Bass kernel for GQA MultiHeadAttention with RoPE.

Problem: B=2, S=2048, D=1024, H=16 Q heads, KVH=4 KV heads, head_dim=64,
causal additive mask, f32.

Sharding: tensor-parallel over heads (TP=4: 4 Q heads + 1 KV head per shard)
x data-parallel over batch (DP=2) = 8 NeuronCores. Wo is sharded on its
input dim; the host sums the 4 partial outputs per batch element.

Design notes (tuned against the TimelineSim cost model, HW-validated):
- ctx matmuls use probs as the STATIONARY operand (out = [128 qpos, 65]
  per k-tile, Ldweights is free) cutting ctx PE cost ~2.3x vs V-stationary,
  and putting the softmax rowsum per-partition: normalization is a single
  DVE reciprocal + tensor_scalar, no cross-partition reductions.
- exp is the only ACT-engine work; all PSUM evictions ride ACT (early,
  while idle) or DVE (GPSIMD cannot touch PSUM on real HW).
- the two heads of each (q-block, head-pair) interleave unit-by-unit so
  ACT always has a ready exp; K/V+Q projections and the output projection
  are chopped into ~0.4us filler thunks drained one-per-attention-unit,
  queued as late as dependencies allow so the late ACT-bound blocks stay
  fed; diag (masked) units run first within each block.
- all four ctx accumulation chains of a head share one 2KB PSUM bank:
  first-emitted matmul start=True zeroes the zero-region, the rest
  accumulate into disjoint addresses, last-emitted carries stop=True.
- bf16 everywhere precision allows (q/k/probs/V/ctxT/Wo/cos/sin/output
  partials); fp32 PSUM accumulation throughout keeps rel err ~4e-3.
"""

import os
import sys

for _p in ("/opt/trn_rl_repo", "/root/.axon_site/_ro/trn_rl_repo"):
    if os.path.isdir(_p) and _p not in sys.path:
        sys.path.insert(0, _p)

import numpy as np
import ml_dtypes

import concourse.bacc as bacc
import concourse.bass as bass
import concourse.tile as tile
from concourse import mybir
from concourse.bass_utils import run_bass_kernel_spmd

F32 = mybir.dt.float32
F32R = mybir.dt.float32r
BF16 = mybir.dt.bfloat16
AF = mybir.ActivationFunctionType

H, KVH, HD = 16, 4, 64
B, S, D = 2, 2048, 1024
TP = 4                      # head-parallel ways
SCALE = HD ** -0.5
NEG = -1e9
NT = S // 128               # 16 kv tiles
NQB = S // 512              # 4 q blocks


def _patch_act_tables():
    """Make Exp resolve only to natural_log_exp_and_others so the
    act-table-load pass emits one load instead of thrashing."""
    from concourse.hw_specs import get_activation_tables
    t = get_activation_tables("gen3")
    for name, fns in t.items():
        if name != "natural_log_exp_and_others":
            fns.discard(AF.Exp)
            fns.discard(AF.Ln)


def _build_nc(causal: bool):
    _patch_act_tables()
    nc = bacc.Bacc()

    hT = nc.declare_dram_parameter("hT", [D, S], BF16, isOutput=False)
    cs128 = nc.declare_dram_parameter("cs128", [128, S], BF16, isOutput=False)
    sn128 = nc.declare_dram_parameter("sn128", [128, S], BF16, isOutput=False)
    wq = nc.declare_dram_parameter("wq", [128, 8, 256], BF16, isOutput=False)
    wkv = nc.declare_dram_parameter("wkv", [128, 8, 128], BF16, isOutput=False)
    wo = nc.declare_dram_parameter("wo", [128, 2, D], BF16, isOutput=False)
    psigT = nc.declare_dram_parameter("psigT", [128, 128], F32R, isOutput=False)
    ident = nc.declare_dram_parameter("ident", [128, 128], F32R, isOutput=False)
    idb = nc.declare_dram_parameter("idb", [128, 128], BF16, isOutput=False)
    m01 = nc.declare_dram_parameter("m01", [128, 128], BF16, isOutput=False)
    outp = nc.declare_dram_parameter("out", [S, D], BF16, isOutput=True)

    with tile.TileContext(nc) as tc:
        with tc.tile_pool(name="hold", bufs=1) as hp:
            # load order matters: K/V projection inputs first so compute can
            # start ASAP; second hidden half + Wo and small consts later
            wkv_sb = hp.tile([128, 8, 128], BF16, name="wkv_sb", tag="wkv_sb")
            nc.sync.dma_start(out=wkv_sb, in_=wkv[:, :, :])
            ht_sb = [hp.tile([128, S], BF16, name=f"ht{c}", tag=f"ht{c}")
                     for c in range(8)]
            for c in range(8):
                eng = nc.sync if c % 2 == 0 else nc.gpsimd
                eng.dma_start(out=ht_sb[c][:, 0:1024],
                              in_=hT[c * 128:(c + 1) * 128, 0:1024])
            psig_sb = hp.tile([128, 128], F32R, name="psig_sb", tag="psig_sb")
            nc.sync.dma_start(out=psig_sb, in_=psigT[:, :])
            cosf_sb = hp.tile([128, S], BF16, name="cosf_sb", tag="cosf_sb")
            sinf_sb = hp.tile([128, S], BF16, name="sinf_sb", tag="sinf_sb")
            nc.gpsimd.dma_start(out=cosf_sb[:, 0:1024], in_=cs128[:, 0:1024])
            nc.sync.dma_start(out=sinf_sb[:, 0:1024], in_=sn128[:, 0:1024])
            id_sb = hp.tile([128, 128], F32R, name="id_sb", tag="id_sb")
            nc.sync.dma_start(out=id_sb, in_=ident[:, :])
            wq_sb = hp.tile([128, 8, 256], BF16, name="wq_sb", tag="wq_sb")
            nc.sync.dma_start(out=wq_sb, in_=wq[:, :, :])
            nc.sync.dma_start(out=cosf_sb[:, 1024:2048], in_=cs128[:, 1024:2048])
            nc.sync.dma_start(out=sinf_sb[:, 1024:2048], in_=sn128[:, 1024:2048])
            m01_sb = hp.tile([128, 128], BF16, name="m01_sb", tag="m01_sb")
            nc.sync.dma_start(out=m01_sb, in_=m01[:, :])
            for c in range(8):
                nc.sync.dma_start(out=ht_sb[c][:, 1024:2048],
                                  in_=hT[c * 128:(c + 1) * 128, 1024:2048])
            idb_sb = hp.tile([128, 128], BF16, name="idb_sb", tag="idb_sb")
            nc.sync.dma_start(out=idb_sb, in_=idb[:, :])
            wo_sb = hp.tile([128, 2, D], BF16, name="wo_sb", tag="wo_sb")
            nc.sync.dma_start(out=wo_sb, in_=wo[:, :, :])

            qTs = [hp.tile([128, S], BF16, name=f"qT{p}", tag=f"qT{p}")
                   for p in range(2)]
            kT = hp.tile([128, S], BF16, name="kTt", tag="kTt")
            vsm = hp.tile([128, NT, 65], BF16, name="vsm", tag="vsm")
            ctxTs = [[hp.tile([128, 512], BF16, name=f"ctxT{c}_{q}",
                              tag=f"ctxT{c}_{q}") for q in range(NQB)]
                     for c in range(2)]

            # ones column (65th) of vsm for the softmax denominator
            nc.vector.memset(vsm[:, :, 64:65], 1.0)

            with tc.tile_pool(name="psS", bufs=1, space="PSUM") as psS, \
                 tc.tile_pool(name="psC", bufs=1, space="PSUM") as psC, \
                 tc.tile_pool(name="psD", bufs=1, space="PSUM") as psD, \
                 tc.tile_pool(name="etp", bufs=1) as etp, \
                 tc.tile_pool(name="sbA", bufs=4) as sbA, \
                 tc.tile_pool(name="sbC", bufs=1) as sbC:

                # ---- filler queue: small PE-work thunks drained one per
                # attention unit so projections/output ride the exp shadow
                filler = []

                def drain(n=1):
                    for _ in range(n):
                        if filler:
                            filler.pop(0)[1]()

                def ensure(label):
                    rest, todo = [], []
                    for it in filler:
                        (todo if it[0] == label else rest).append(it)
                    filler[:] = rest
                    for _, th in todo:
                        th()

                def drain_all():
                    while filler:
                        filler.pop(0)[1]()

                # deferred phase-C emission (transposes + ctxT evicts)
                pending_c = []

                def flush_pending():
                    while pending_c:
                        pending_c.pop(0)()

                # ---------------- Phase A: projections + rope ----------------
                def queue_q_sc(pp, sc):
                    label = f"q{pp}sc{sc}"
                    csl = slice(512 * sc, 512 * sc + 512)
                    box = {}

                    def proj_a():
                        ps_q = psD.tile([128, 512], F32, name="ps_q",
                                        tag="ps_d", bufs=2)
                        for dc in range(4):
                            nc.tensor.matmul(
                                ps_q,
                                wq_sb[:, dc, 128 * pp:128 * pp + 128],
                                ht_sb[dc][:, csl],
                                start=(dc == 0), stop=False)
                        box["ps"] = ps_q

                    def proj_b():
                        ps_q = box["ps"]
                        for dc in range(4, 8):
                            nc.tensor.matmul(
                                ps_q,
                                wq_sb[:, dc, 128 * pp:128 * pp + 128],
                                ht_sb[dc][:, csl],
                                start=False, stop=(dc == 7))

                    def rope():
                        ps_q = box["ps"]
                        qraw = sbA.tile([128, 512], F32R, name="qraw",
                                        tag="qraw")
                        nc.vector.tensor_copy(qraw, ps_q)
                        ps_rot = psD.tile([128, 512], F32, name="ps_rot",
                                          tag="ps_d", bufs=2)
                        nc.tensor.matmul(ps_rot, psig_sb.bitcast(F32R),
                                         qraw.bitcast(F32R),
                                         start=True, stop=True)
                        qc = sbA.tile([128, 512], F32, name="qc", tag="qc")
                        nc.gpsimd.tensor_mul(qc, qraw.bitcast(F32),
                                             cosf_sb[:, csl])
                        rtmp = sbA.tile([128, 512], F32, name="rtmp",
                                        tag="rtmp")
                        nc.vector.tensor_mul(rtmp, ps_rot, sinf_sb[:, csl])
                        nc.gpsimd.tensor_add(qTs[pp][:, csl], qc, rtmp)

                    filler.append((label, proj_a))
                    filler.append((label, proj_b))
                    filler.append((label, rope))

                def queue_kv_sc(sc, direct=False):
                    # K/V: kvT = [Wk|Wv].T @ h.T -> K rows 0:64, V rows 64:128
                    label = f"kvsc{sc}"
                    csl = slice(512 * sc, 512 * sc + 512)
                    box = {}

                    def proj_a():
                        ps_kv = psD.tile([128, 512], F32, name="ps_kv",
                                         tag="ps_d", bufs=2)
                        for dc in range(4):
                            nc.tensor.matmul(
                                ps_kv,
                                wkv_sb[:, dc, :],
                                ht_sb[dc][:, csl],
                                start=(dc == 0), stop=False)
                        box["ps"] = ps_kv

                    def proj_b():
                        ps_kv = box["ps"]
                        for dc in range(4, 8):
                            nc.tensor.matmul(
                                ps_kv,
                                wkv_sb[:, dc, :],
                                ht_sb[dc][:, csl],
                                start=False, stop=(dc == 7))

                    def krope():
                        kvraw = sbA.tile([128, 512], F32R, name="kvraw",
                                         tag="kvraw")
                        nc.vector.tensor_copy(kvraw, box["ps"])
                        box["kvraw"] = kvraw
                        # rope on K rows
                        ps_krot = psD.tile([128, 512], F32, name="ps_krot",
                                           tag="ps_d", bufs=2)[0:64, :]
                        nc.tensor.matmul(ps_krot,
                                         psig_sb[0:64, 0:64].bitcast(F32R),
                                         kvraw[0:64, :].bitcast(F32R),
                                         start=True, stop=True)
                        kc = sbA.tile([64, 512], F32, name="kc", tag="kc")
                        nc.gpsimd.tensor_mul(kc, kvraw[0:64, :].bitcast(F32),
                                             cosf_sb[0:64, csl])
                        ktmp = sbA.tile([64, 512], F32, name="ktmp", tag="ktmp")
                        nc.vector.tensor_mul(ktmp, ps_krot, sinf_sb[0:64, csl])
                        nc.gpsimd.tensor_add(kT[0:64, csl], kc, ktmp)
                        # duplicate roped K to partitions 64:128 (engines
                        # cannot cross partitions; DMA can)
                        nc.sync.dma_start(out=kT[64:128, csl],
                                          in_=kT[0:64, csl])

                    def vt(pair):
                        # V: transpose 128-seq tiles into vsm (seq-major)
                        kvraw = box["kvraw"]
                        for tt in (2 * pair, 2 * pair + 1):
                            ti = 4 * sc + tt
                            ps_v = psD.tile([128, 512], F32, name="ps_v",
                                            tag="ps_d", bufs=2)[:, 0:64]
                            nc.tensor.matmul(
                                ps_v.bitcast(F32R),
                                kvraw[64:128, 128 * tt:128 * tt + 128].bitcast(F32R),
                                id_sb[64:128, 0:64].bitcast(F32R),
                                start=True, stop=True, is_transpose=True)
                            nc.vector.tensor_copy(vsm[:, ti, 0:64], ps_v)

                    thunks = [proj_a, proj_b, krope,
                              lambda: vt(0), lambda: vt(1)]
                    if direct:
                        for th in thunks:
                            th()
                    else:
                        for th in thunks:
                            filler.append((label, th))

                def queue_phase_d(dq, qts=None):
                    label = f"pd{dq}"
                    for qt in (qts if qts is not None
                               else range(4 * dq, 4 * dq + 4)):
                        ct0 = ctxTs[0][qt // 4]
                        ct1 = ctxTs[1][qt // 4]
                        col = 128 * (qt % 4)
                        for nb in range(2):
                            def th(_qt=qt, _nb=nb, _ct0=ct0, _ct1=ct1,
                                   _col=col):
                                ps_o = psD.tile([128, 512], F32, name="ps_o",
                                                tag="ps_d", bufs=2)
                                nc.tensor.matmul(
                                    ps_o, _ct0[:, _col:_col + 128],
                                    wo_sb[:, 0, 512 * _nb:512 * _nb + 512],
                                    start=True, stop=False)
                                nc.tensor.matmul(
                                    ps_o, _ct1[:, _col:_col + 128],
                                    wo_sb[:, 1, 512 * _nb:512 * _nb + 512],
                                    start=False, stop=True)
                                ost = sbC.tile([128, 512], BF16, name="ost",
                                               tag="ost", bufs=6)
                                nc.vector.tensor_copy(ost, ps_o)
                                nc.sync.dma_start(
                                    out=outp[128 * _qt:128 * _qt + 128,
                                             512 * _nb:512 * _nb + 512],
                                    in_=ost)
                            filler.append((label, th))

                def build_head(qb, hh, sp):
                    h = 2 * sp + hh
                    off = 64 * (h % 2)
                    pp = h // 2
                    ps_ctx = psC.tile([128, 4, 65], F32, name=f"ps_ctx{hh}",
                                      tag="ps_ctx", bufs=2)
                    nfull = (4 * qb) if causal else NT
                    lastki = (4 * qb + 3) if causal else (NT - 1)
                    units = []

                    ctx_total = (16 * qb + 10) if causal else 64
                    ctx_cnt = [0]

                    def ctx_mms(et_ap, ki, jlist, base_idx=0):
                        # et_ap: probs chunk row; one [q,65] matmul per
                        # q-subtile j (chunk at base_idx+n within et_ap).
                        # All 4 q-subtile chains share one psum bank: the
                        # FIRST EMITTED matmul's start=True lazily zeroes
                        # the whole 2KB zero-region, every later matmul
                        # accumulates (disjoint addresses read as zero), and
                        # the LAST one closes the group with stop=True.
                        for idx, j in enumerate(jlist):
                            start = (ctx_cnt[0] == 0)
                            stop = (ctx_cnt[0] == ctx_total - 1)
                            ctx_cnt[0] += 1
                            o = 128 * (base_idx + idx)
                            nc.tensor.matmul(
                                ps_ctx[:, j, :],
                                et_ap[:, o:o + 128],
                                vsm[:, ki, 0:65],
                                start=start, stop=stop)

                    def mk_pair(kp):
                        box = {}

                        def s():
                            ps_s = psS.tile([128, 1024], F32, name="ps_s",
                                            tag="ps_s", bufs=2)
                            for jj in range(2):
                                ki = kp + jj
                                nc.tensor.matmul(
                                    ps_s[:, 512 * jj:512 * jj + 512],
                                    kT[off:off + 64,
                                       128 * ki:128 * ki + 128],
                                    qTs[pp][off:off + 64,
                                            512 * qb:512 * qb + 512],
                                    start=True, stop=True)
                            box["ps"] = ps_s

                        def ec():
                            et = etp.tile([128, 1024], BF16, name="et",
                                          tag="et", bufs=8)
                            nc.scalar.activation(et, box["ps"], AF.Exp,
                                                 scale=SCALE)
                            for jj in range(2):
                                ki = kp + jj
                                ctx_mms(et[:, 512 * jj:512 * jj + 512],
                                        ki, [0, 1, 2, 3])
                        return (s, ec)

                    def mk_diag(which):
                        # which=0: j=0 (span 512 @0) + j=1 (span 384 @512)
                        # which=1: j=2 (span 256 @0) + j=3 (span 128 @256)
                        box = {}
                        js = (0, 1) if which == 0 else (2, 3)
                        offs = (0, 512) if which == 0 else (0, 256)

                        def s():
                            ps_s = psS.tile([128, 1024], F32, name="ps_dg",
                                            tag="ps_s", bufs=2)
                            for j, o in zip(js, offs):
                                ki = 4 * qb + j
                                span = 512 - 128 * j
                                nc.tensor.matmul(
                                    ps_s[:, o:o + span],
                                    kT[off:off + 64,
                                       128 * ki:128 * ki + 128],
                                    qTs[pp][off:off + 64,
                                            512 * qb + 128 * j:
                                            512 * (qb + 1)],
                                    start=True, stop=True)
                            box["ps"] = ps_s

                        def ec():
                            wid = 896 if which == 0 else 384
                            et = etp.tile([128, 1024], BF16, name="etd",
                                          tag="et", bufs=8)
                            nc.scalar.activation(et[:, 0:wid],
                                                 box["ps"][:, 0:wid],
                                                 AF.Exp, scale=SCALE)
                            # mask the diagonal 128x128 chunk of each j
                            for j, o in zip(js, offs):
                                eng = nc.vector
                                eng.tensor_mul(et[:, o:o + 128],
                                               et[:, o:o + 128], m01_sb)
                            for j, o in zip(js, offs):
                                ki = 4 * qb + j
                                span = et[:, o:o + 512 - 128 * j]
                                jl = list(range(j, 4))
                                # unmasked q-subtiles first; the masked
                                # diagonal chunk (qt==j) last
                                ctx_mms(span, ki, jl[1:], base_idx=1)
                                ctx_mms(span, ki, jl[:1], base_idx=0)
                        return (s, ec)

                    if causal:
                        units.append(mk_diag(0))
                        units.append(mk_diag(1))
                    for kp in range(0, nfull, 2):
                        units.append(mk_pair(kp))

                    def phase_c(ctxns_h):
                        rc = sbC.tile([128, 4, 1], F32, name="rc", tag="rc",
                                      bufs=4)
                        nc.vector.reciprocal(rc, ps_ctx[:, :, 64:65])
                        for j in range(4):
                            cn = sbC.tile([128, 64], BF16, name="ctxn",
                                          tag="ctxn", bufs=16)
                            nc.vector.tensor_scalar_mul(
                                cn, ps_ctx[:, j, 0:64], rc[:, j, 0:1])
                            ctxns_h[j] = cn

                    return units, phase_c

                def emit_bc(qb, sp, post_flush=None, queue_fillers=None):
                    # attention for one (q block, head pair); the two heads'
                    # unit streams interleave so ACT always has a ready exp
                    ensure(f"q{sp}sc{qb}")
                    if qb > 0:
                        ensure(f"kvsc{qb}")
                    ctxns = [[None] * 4 for _ in range(2)]
                    u0, pc0 = build_head(qb, 0, sp)
                    u1, pc1 = build_head(qb, 1, sp)
                    n = len(u0)
                    u0[0][0]()
                    if post_flush is not None:
                        flush_pending()
                        post_flush()
                    if queue_fillers is not None:
                        queue_fillers()
                    u1[0][0]()
                    drain(1)
                    for i in range(n):
                        if i == 1:
                            # flush the previous block's transposes one
                            # round in, when their normalize chain is done
                            flush_pending()
                        if i + 1 < n:
                            u0[i + 1][0]()
                            drain(1)
                        u0[i][1]()
                        if i + 1 < n:
                            u1[i + 1][0]()
                            drain(1)
                        u1[i][1]()
                    pc0(ctxns[0])
                    pc1(ctxns[1])

                    def do_transposes(_sp=sp, _qb=qb, _ctxns=ctxns):
                        # 8 transposed chunks share one psum zero-region:
                        # first start=True zeroes it, the rest accumulate
                        # into disjoint (zeroed) addresses
                        ps_t = psD.tile([128, 512], F32, name="ps_t",
                                        tag="ps_d", bufs=2).bitcast(BF16)
                        for hh in range(2):
                            base = 64 * hh
                            for j in range(4):
                                nc.tensor.matmul(
                                    ps_t[base:base + 64,
                                         128 * j:128 * j + 128],
                                    _ctxns[hh][j], idb_sb[:, 0:128],
                                    start=(j == 0), stop=(j == 3),
                                    is_transpose=True)
                        for hh in range(2):
                            nc.vector.tensor_copy(
                                ctxTs[hh][_qb][64 * _sp:64 * _sp + 64, :],
                                ps_t[64 * hh:64 * hh + 64, 0:512])
                    pending_c.append(do_transposes)

                # ---- global emission order ----
                queue_kv_sc(0, direct=True)
                queue_q_sc(0, 0)
                ensure("q0sc0")
                queue_q_sc(1, 0)
                queue_kv_sc(1)
                queue_q_sc(0, 1)
                emit_bc(0, 0)
                queue_kv_sc(2)
                queue_q_sc(1, 1)
                emit_bc(0, 1)
                queue_q_sc(0, 2)
                queue_q_sc(1, 2)
                emit_bc(1, 0)
                queue_kv_sc(3)
                emit_bc(1, 1)

                def q20_fill():
                    queue_q_sc(0, 3)
                    queue_q_sc(1, 3)
                    queue_phase_d(0)
                emit_bc(2, 0, queue_fillers=q20_fill)
                emit_bc(2, 1, queue_fillers=lambda: queue_phase_d(1))
                emit_bc(3, 0, post_flush=None)
                emit_bc(3, 1, queue_fillers=lambda: queue_phase_d(2))
                drain_all()
                flush_pending()
                # tail: last q block's output projection, double-width psum
                # slots from the now-idle attention ring for deep pipelining
                for qt in range(12, 16):
                    ps_o = psS.tile([128, 1024], F32, name="ps_ow",
                                    tag="ps_s", bufs=2)
                    col = 128 * (qt % 4)
                    for nb in range(2):
                        for c in range(2):
                            nc.tensor.matmul(
                                ps_o[:, 512 * nb:512 * nb + 512],
                                ctxTs[c][3][:, col:col + 128],
                                wo_sb[:, c, 512 * nb:512 * nb + 512],
                                start=(c == 0), stop=(c == 1))
                    ost = sbC.tile([128, 1024], BF16, name="ostw",
                                   tag="ostw", bufs=4)
                    nc.scalar.copy(ost[:, 0:512], ps_o[:, 0:512])
                    nc.vector.tensor_copy(ost[:, 512:1024], ps_o[:, 512:1024])
                    nc.sync.dma_start(
                        out=outp[128 * qt:128 * qt + 128, :], in_=ost)

    nc.compile()
    return nc


_NC_CACHE = {}


def _get_nc(causal: bool):
    if causal not in _NC_CACHE:
        _NC_CACHE[causal] = _build_nc(causal)
    return _NC_CACHE[causal]


def _host_consts():
    p = np.zeros((128, 128), np.float32)
    idx = np.arange(0, 128, 2)
    p[idx, idx + 1] = -1.0
    p[idx + 1, idx] = 1.0
    psigT = np.ascontiguousarray(p.T)
    ident = np.eye(128, dtype=np.float32)
    ident[64:128, 0:64] = np.eye(64, dtype=np.float32)
    idb = np.eye(128, dtype=ml_dtypes.bfloat16)
    m01 = (np.arange(128)[None, :] >= np.arange(128)[:, None])
    m01 = m01.astype(ml_dtypes.bfloat16)
    return psigT, ident, idb, m01


def _numpy_reference(hidden_states, cos, sin, attention_mask, Wq, Wk, Wv, Wo):
    """Generic-mask fallback, pure numpy port of the reference."""
    GROUPS = H // KVH

    def rope(x, c, s):
        c = c[:, None, :, :]
        s = s[:, None, :, :]
        x1, x2 = x[..., ::2], x[..., 1::2]
        xr = np.stack([x1 * c - x2 * s, x1 * s + x2 * c], axis=-1)
        return xr.reshape(x.shape)

    b, sq, d = hidden_states.shape
    q = (hidden_states @ Wq).reshape(b, sq, H, HD).transpose(0, 2, 1, 3)
    k = (hidden_states @ Wk).reshape(b, sq, KVH, HD).transpose(0, 2, 1, 3)
    v = (hidden_states @ Wv).reshape(b, sq, KVH, HD).transpose(0, 2, 1, 3)
    q = rope(q, cos, sin)
    k = rope(k, cos, sin)
    k = np.repeat(k, GROUPS, axis=1)
    v = np.repeat(v, GROUPS, axis=1)
    out = np.zeros((b, sq, d), np.float32)
    for bi in range(b):
        for hi in range(H):
            sc = (q[bi, hi] @ k[bi, hi].T) * SCALE + attention_mask[0, 0]
            sc = sc - sc.max(axis=-1, keepdims=True)
            e = np.exp(sc)
            pr = e / e.sum(axis=-1, keepdims=True)
            ctx = pr @ v[bi, hi]
            out[bi] += ctx @ Wo[hi * HD:(hi + 1) * HD]
    return out


def _make_in_maps(hs, cos, sin, Wq, Wk, Wv, Wo):
    psigT, ident, idb, m01 = _host_consts()
    chan_half = (np.arange(64) // 2)

    in_maps = []
    for core in range(8):
        b, t = core // TP, core % TP
        hT = np.ascontiguousarray(hs[b].T).astype(ml_dtypes.bfloat16)
        cs64v = np.ascontiguousarray(cos[b].T[chan_half, :])
        sn64v = np.ascontiguousarray(sin[b].T[chan_half, :])
        cs128v = np.ascontiguousarray(np.concatenate([cs64v, cs64v], axis=0)).astype(ml_dtypes.bfloat16)
        sn128v = np.ascontiguousarray(np.concatenate([sn64v, sn64v], axis=0)).astype(ml_dtypes.bfloat16)
        wq_s = Wq[:, t * 256:(t + 1) * 256].reshape(8, 128, 256)
        wq_s = np.ascontiguousarray(
            wq_s.transpose(1, 0, 2)).astype(ml_dtypes.bfloat16)
        wkv_s = np.concatenate([Wk[:, t * 64:(t + 1) * 64],
                                Wv[:, t * 64:(t + 1) * 64]],
                               axis=1).reshape(8, 128, 128)
        wkv_s = np.ascontiguousarray(
            wkv_s.transpose(1, 0, 2)).astype(ml_dtypes.bfloat16)
        wo_s = Wo[t * 256:(t + 1) * 256]
        # ctxT channel order per chunk: c0 = [h0|h2], c1 = [h1|h3]
        wo_p = np.concatenate([wo_s[0:64], wo_s[128:192],
                               wo_s[64:128], wo_s[192:256]],
                              axis=0).reshape(2, 128, 1024)
        wo_p = np.ascontiguousarray(
            wo_p.transpose(1, 0, 2)).astype(ml_dtypes.bfloat16)
        in_maps.append({
            "hT": hT, "cs128": cs128v, "sn128": sn128v,
            "wq": wq_s, "wkv": wkv_s, "wo": wo_p,
            "psigT": psigT, "ident": ident, "idb": idb, "m01": m01,
        })
    return in_maps


def kernel(**inputs) -> np.ndarray:
    hs = np.asarray(inputs["hidden_states"], np.float32)
    cos = np.asarray(inputs["cos"], np.float32)
    sin = np.asarray(inputs["sin"], np.float32)
    mask = np.asarray(inputs["attention_mask"], np.float32)
    Wq = np.asarray(inputs["Wq"], np.float32)
    Wk = np.asarray(inputs["Wk"], np.float32)
    Wv = np.asarray(inputs["Wv"], np.float32)
    Wo = np.asarray(inputs["Wo"], np.float32)

    m = mask.reshape(S, S)
    tril = np.tril(np.ones((S, S), dtype=bool))
    causal_ref = np.where(tril, np.float32(0.0), np.float32(NEG))
    if np.array_equal(m, causal_ref):
        causal = True
    elif not m.any():
        causal = False
    else:
        return _numpy_reference(hs, cos, sin, mask, Wq, Wk, Wv, Wo)

    nc = _get_nc(causal)
    in_maps = _make_in_maps(hs, cos, sin, Wq, Wk, Wv, Wo)
    res = run_bass_kernel_spmd(nc, in_maps, core_ids=list(range(8)))
    out = np.zeros((B, S, D), np.float32)
    for core in range(8):
        out[core // TP] += res.results[core]["out"].astype(np.float32)
    return out


# revision 13
# speedup vs baseline: 1.0346x; 1.0157x over previous
"""Self-contained Trainium2 Bass kernel for GQA MultiHeadAttention with RoPE.

Problem: B=2, S=2048, D=1024, H=16 Q heads, KVH=4 KV heads, head_dim=64,
causal additive mask, f32.

Sharding: tensor-parallel over heads (TP=4: 4 Q heads + 1 KV head per shard)
x data-parallel over batch (DP=2) = 8 NeuronCores. Wo is sharded on its
input dim; the host sums the 4 partial outputs per batch element.

Design notes (tuned against the TimelineSim cost model, HW-validated):
- ctx matmuls use probs as the STATIONARY operand (out = [128 qpos, 65]
  per k-tile, Ldweights is free) cutting ctx PE cost ~2.3x vs V-stationary,
  and putting the softmax rowsum per-partition: normalization is a single
  DVE reciprocal + tensor_scalar, no cross-partition reductions.
- exp is the only ACT-engine work; all PSUM evictions ride ACT (early,
  while idle) or DVE (GPSIMD cannot touch PSUM on real HW).
- the two heads of each (q-block, head-pair) interleave unit-by-unit so
  ACT always has a ready exp; K/V+Q projections and the output projection
  are chopped into ~0.4us filler thunks drained one-per-attention-unit,
  queued as late as dependencies allow so the late ACT-bound blocks stay
  fed; diag (masked) units run first within each block.
- all four ctx accumulation chains of a head share one 2KB PSUM bank:
  first-emitted matmul start=True zeroes the zero-region, the rest
  accumulate into disjoint addresses, last-emitted carries stop=True.
- bf16 everywhere precision allows (q/k/probs/V/ctxT/Wo/cos/sin/output
  partials); fp32 PSUM accumulation throughout keeps rel err ~4e-3.
"""

import os
import sys

for _p in ("/opt/trn_rl_repo", "/root/.axon_site/_ro/trn_rl_repo"):
    if os.path.isdir(_p) and _p not in sys.path:
        sys.path.insert(0, _p)

import numpy as np
import ml_dtypes

import concourse.bacc as bacc
import concourse.bass as bass
import concourse.tile as tile
from concourse import mybir
from concourse.bass_utils import run_bass_kernel_spmd

F32 = mybir.dt.float32
F32R = mybir.dt.float32r
BF16 = mybir.dt.bfloat16
AF = mybir.ActivationFunctionType

H, KVH, HD = 16, 4, 64
B, S, D = 2, 2048, 1024
TP = 4                      # head-parallel ways
SCALE = HD ** -0.5
NEG = -1e9
NT = S // 128               # 16 kv tiles
NQB = S // 512              # 4 q blocks


def _patch_act_tables():
    """Make Exp resolve only to natural_log_exp_and_others so the
    act-table-load pass emits one load instead of thrashing."""
    from concourse.hw_specs import get_activation_tables
    t = get_activation_tables("gen3")
    for name, fns in t.items():
        if name != "natural_log_exp_and_others":
            fns.discard(AF.Exp)
            fns.discard(AF.Ln)


def _build_nc(causal: bool):
    _patch_act_tables()
    nc = bacc.Bacc()

    hT = nc.declare_dram_parameter("hT", [D, S], BF16, isOutput=False)
    cs128 = nc.declare_dram_parameter("cs128", [128, S], BF16, isOutput=False)
    sn128 = nc.declare_dram_parameter("sn128", [128, S], BF16, isOutput=False)
    wq = nc.declare_dram_parameter("wq", [128, 8, 256], BF16, isOutput=False)
    wkv = nc.declare_dram_parameter("wkv", [128, 8, 128], BF16, isOutput=False)
    wo = nc.declare_dram_parameter("wo", [128, 2, D], BF16, isOutput=False)
    psigT = nc.declare_dram_parameter("psigT", [128, 128], F32R, isOutput=False)
    ident = nc.declare_dram_parameter("ident", [128, 128], F32R, isOutput=False)
    idb = nc.declare_dram_parameter("idb", [128, 128], BF16, isOutput=False)
    m01 = nc.declare_dram_parameter("m01", [128, 128], BF16, isOutput=False)
    outp = nc.declare_dram_parameter("out", [S, D], BF16, isOutput=True)

    with tile.TileContext(nc) as tc:
        with tc.tile_pool(name="hold", bufs=1) as hp:
            # load order matters: K/V projection inputs first so compute can
            # start ASAP; second hidden half + Wo and small consts later
            wkv_sb = hp.tile([128, 8, 128], BF16, name="wkv_sb", tag="wkv_sb")
            nc.sync.dma_start(out=wkv_sb, in_=wkv[:, :, :])
            ht_sb = [hp.tile([128, S], BF16, name=f"ht{c}", tag=f"ht{c}")
                     for c in range(8)]
            for c in range(8):
                eng = nc.sync if c % 2 == 0 else nc.gpsimd
                eng.dma_start(out=ht_sb[c][:, 0:1024],
                              in_=hT[c * 128:(c + 1) * 128, 0:1024])
            psig_sb = hp.tile([128, 128], F32R, name="psig_sb", tag="psig_sb")
            nc.sync.dma_start(out=psig_sb, in_=psigT[:, :])
            cosf_sb = hp.tile([128, S], BF16, name="cosf_sb", tag="cosf_sb")
            sinf_sb = hp.tile([128, S], BF16, name="sinf_sb", tag="sinf_sb")
            nc.gpsimd.dma_start(out=cosf_sb[:, 0:1024], in_=cs128[:, 0:1024])
            nc.sync.dma_start(out=sinf_sb[:, 0:1024], in_=sn128[:, 0:1024])
            id_sb = hp.tile([128, 128], F32R, name="id_sb", tag="id_sb")
            nc.sync.dma_start(out=id_sb, in_=ident[:, :])
            wq_sb = hp.tile([128, 8, 256], BF16, name="wq_sb", tag="wq_sb")
            nc.sync.dma_start(out=wq_sb, in_=wq[:, :, :])
            nc.sync.dma_start(out=cosf_sb[:, 1024:2048], in_=cs128[:, 1024:2048])
            nc.sync.dma_start(out=sinf_sb[:, 1024:2048], in_=sn128[:, 1024:2048])
            m01_sb = hp.tile([128, 128], BF16, name="m01_sb", tag="m01_sb")
            nc.sync.dma_start(out=m01_sb, in_=m01[:, :])
            for c in range(8):
                nc.sync.dma_start(out=ht_sb[c][:, 1024:2048],
                                  in_=hT[c * 128:(c + 1) * 128, 1024:2048])
            idb_sb = hp.tile([128, 128], BF16, name="idb_sb", tag="idb_sb")
            nc.sync.dma_start(out=idb_sb, in_=idb[:, :])
            wo_sb = hp.tile([128, 2, D], BF16, name="wo_sb", tag="wo_sb")
            nc.sync.dma_start(out=wo_sb, in_=wo[:, :, :])

            qTs = [hp.tile([128, S], BF16, name=f"qT{p}", tag=f"qT{p}")
                   for p in range(2)]
            kT = hp.tile([128, S], BF16, name="kTt", tag="kTt")
            vsm = hp.tile([128, NT, 65], BF16, name="vsm", tag="vsm")
            ctxTs = [[hp.tile([128, 512], BF16, name=f"ctxT{c}_{q}",
                              tag=f"ctxT{c}_{q}") for q in range(NQB)]
                     for c in range(2)]

            # ones column (65th) of vsm for the softmax denominator
            nc.vector.memset(vsm[:, :, 64:65], 1.0)

            with tc.tile_pool(name="psS", bufs=1, space="PSUM") as psS, \
                 tc.tile_pool(name="psC", bufs=1, space="PSUM") as psC, \
                 tc.tile_pool(name="psD", bufs=1, space="PSUM") as psD, \
                 tc.tile_pool(name="etp", bufs=1) as etp, \
                 tc.tile_pool(name="sbA", bufs=4) as sbA, \
                 tc.tile_pool(name="sbC", bufs=1) as sbC:

                # ---- filler queue: small PE-work thunks drained one per
                # attention unit so projections/output ride the exp shadow
                filler = []

                def drain(n=1):
                    for _ in range(n):
                        if filler:
                            filler.pop(0)[1]()

                def ensure(label):
                    rest, todo = [], []
                    for it in filler:
                        (todo if it[0] == label else rest).append(it)
                    filler[:] = rest
                    for _, th in todo:
                        th()

                def drain_all():
                    while filler:
                        filler.pop(0)[1]()

                # deferred phase-C emission (transposes + ctxT evicts)
                pending_c = []

                def flush_pending():
                    while pending_c:
                        pending_c.pop(0)()

                # ---------------- Phase A: projections + rope ----------------
                def queue_q_sc(pp, sc):
                    label = f"q{pp}sc{sc}"
                    csl = slice(512 * sc, 512 * sc + 512)
                    box = {}

                    def proj_a():
                        ps_q = psD.tile([128, 512], F32, name="ps_q",
                                        tag="ps_d", bufs=2)
                        for dc in range(4):
                            nc.tensor.matmul(
                                ps_q,
                                wq_sb[:, dc, 128 * pp:128 * pp + 128],
                                ht_sb[dc][:, csl],
                                start=(dc == 0), stop=False)
                        box["ps"] = ps_q

                    def proj_b():
                        ps_q = box["ps"]
                        for dc in range(4, 8):
                            nc.tensor.matmul(
                                ps_q,
                                wq_sb[:, dc, 128 * pp:128 * pp + 128],
                                ht_sb[dc][:, csl],
                                start=False, stop=(dc == 7))

                    def rope():
                        ps_q = box["ps"]
                        qraw = sbA.tile([128, 512], F32R, name="qraw",
                                        tag="qraw")
                        nc.vector.tensor_copy(qraw, ps_q)
                        ps_rot = psD.tile([128, 512], F32, name="ps_rot",
                                          tag="ps_d", bufs=2)
                        nc.tensor.matmul(ps_rot, psig_sb.bitcast(F32R),
                                         qraw.bitcast(F32R),
                                         start=True, stop=True)
                        qc = sbA.tile([128, 512], F32, name="qc", tag="qc")
                        nc.gpsimd.tensor_mul(qc, qraw.bitcast(F32),
                                             cosf_sb[:, csl])
                        rtmp = sbA.tile([128, 512], F32, name="rtmp",
                                        tag="rtmp")
                        nc.vector.tensor_mul(rtmp, ps_rot, sinf_sb[:, csl])
                        nc.gpsimd.tensor_add(qTs[pp][:, csl], qc, rtmp)

                    filler.append((label, proj_a))
                    filler.append((label, proj_b))
                    filler.append((label, rope))

                def queue_kv_sc(sc, direct=False):
                    # K/V: kvT = [Wk|Wv].T @ h.T -> K rows 0:64, V rows 64:128
                    label = f"kvsc{sc}"
                    csl = slice(512 * sc, 512 * sc + 512)
                    box = {}

                    def proj_a():
                        ps_kv = psD.tile([128, 512], F32, name="ps_kv",
                                         tag="ps_d", bufs=2)
                        for dc in range(4):
                            nc.tensor.matmul(
                                ps_kv,
                                wkv_sb[:, dc, :],
                                ht_sb[dc][:, csl],
                                start=(dc == 0), stop=False)
                        box["ps"] = ps_kv

                    def proj_b():
                        ps_kv = box["ps"]
                        for dc in range(4, 8):
                            nc.tensor.matmul(
                                ps_kv,
                                wkv_sb[:, dc, :],
                                ht_sb[dc][:, csl],
                                start=False, stop=(dc == 7))

                    def krope():
                        kvraw = sbA.tile([128, 512], F32R, name="kvraw",
                                         tag="kvraw")
                        nc.vector.tensor_copy(kvraw, box["ps"])
                        box["kvraw"] = kvraw
                        # rope on K rows
                        ps_krot = psD.tile([128, 512], F32, name="ps_krot",
                                           tag="ps_d", bufs=2)[0:64, :]
                        nc.tensor.matmul(ps_krot,
                                         psig_sb[0:64, 0:64].bitcast(F32R),
                                         kvraw[0:64, :].bitcast(F32R),
                                         start=True, stop=True)
                        kc = sbA.tile([64, 512], F32, name="kc", tag="kc")
                        nc.gpsimd.tensor_mul(kc, kvraw[0:64, :].bitcast(F32),
                                             cosf_sb[0:64, csl])
                        ktmp = sbA.tile([64, 512], F32, name="ktmp", tag="ktmp")
                        nc.vector.tensor_mul(ktmp, ps_krot, sinf_sb[0:64, csl])
                        nc.gpsimd.tensor_add(kT[0:64, csl], kc, ktmp)
                        # duplicate roped K to partitions 64:128 (engines
                        # cannot cross partitions; DMA can)
                        nc.sync.dma_start(out=kT[64:128, csl],
                                          in_=kT[0:64, csl])

                    def vt(pair):
                        # V: transpose 128-seq tiles into vsm (seq-major)
                        kvraw = box["kvraw"]
                        for tt in (2 * pair, 2 * pair + 1):
                            ti = 4 * sc + tt
                            ps_v = psD.tile([128, 512], F32, name="ps_v",
                                            tag="ps_d", bufs=2)[:, 0:64]
                            nc.tensor.matmul(
                                ps_v.bitcast(F32R),
                                kvraw[64:128, 128 * tt:128 * tt + 128].bitcast(F32R),
                                id_sb[64:128, 0:64].bitcast(F32R),
                                start=True, stop=True, is_transpose=True)
                            nc.vector.tensor_copy(vsm[:, ti, 0:64], ps_v)

                    thunks = [proj_a, proj_b, krope,
                              lambda: vt(0), lambda: vt(1)]
                    if direct:
                        for th in thunks:
                            th()
                    else:
                        for th in thunks:
                            filler.append((label, th))

                def queue_phase_d(dq, qts=None):
                    label = f"pd{dq}"
                    for qt in (qts if qts is not None
                               else range(4 * dq, 4 * dq + 4)):
                        ct0 = ctxTs[0][qt // 4]
                        ct1 = ctxTs[1][qt // 4]
                        col = 128 * (qt % 4)
                        for nb in range(2):
                            def th(_qt=qt, _nb=nb, _ct0=ct0, _ct1=ct1,
                                   _col=col):
                                ps_o = psD.tile([128, 512], F32, name="ps_o",
                                                tag="ps_d", bufs=2)
                                nc.tensor.matmul(
                                    ps_o, _ct0[:, _col:_col + 128],
                                    wo_sb[:, 0, 512 * _nb:512 * _nb + 512],
                                    start=True, stop=False)
                                nc.tensor.matmul(
                                    ps_o, _ct1[:, _col:_col + 128],
                                    wo_sb[:, 1, 512 * _nb:512 * _nb + 512],
                                    start=False, stop=True)
                                ost = sbC.tile([128, 512], BF16, name="ost",
                                               tag="ost", bufs=6)
                                nc.vector.tensor_copy(ost, ps_o)
                                nc.sync.dma_start(
                                    out=outp[128 * _qt:128 * _qt + 128,
                                             512 * _nb:512 * _nb + 512],
                                    in_=ost)
                            filler.append((label, th))

                def build_head(qb, hh, sp):
                    h = 2 * sp + hh
                    off = 64 * (h % 2)
                    pp = h // 2
                    ps_ctx = psC.tile([128, 4, 65], F32, name=f"ps_ctx{hh}",
                                      tag="ps_ctx", bufs=2)
                    nfull = (4 * qb) if causal else NT
                    lastki = (4 * qb + 3) if causal else (NT - 1)
                    units = []

                    ctx_total = (16 * qb + 10) if causal else 64
                    ctx_cnt = [0]

                    def ctx_mms(et_ap, ki, jlist, base_idx=0):
                        # et_ap: probs chunk row; one [q,65] matmul per
                        # q-subtile j (chunk at base_idx+n within et_ap).
                        # All 4 q-subtile chains share one psum bank: the
                        # FIRST EMITTED matmul's start=True lazily zeroes
                        # the whole 2KB zero-region, every later matmul
                        # accumulates (disjoint addresses read as zero), and
                        # the LAST one closes the group with stop=True.
                        for idx, j in enumerate(jlist):
                            start = (ctx_cnt[0] == 0)
                            stop = (ctx_cnt[0] == ctx_total - 1)
                            ctx_cnt[0] += 1
                            o = 128 * (base_idx + idx)
                            nc.tensor.matmul(
                                ps_ctx[:, j, :],
                                et_ap[:, o:o + 128],
                                vsm[:, ki, 0:65],
                                start=start, stop=stop)

                    def mk_pair(kp):
                        box = {}

                        def s():
                            ps_s = psS.tile([128, 1024], F32, name="ps_s",
                                            tag="ps_s", bufs=2)
                            for jj in range(2):
                                ki = kp + jj
                                nc.tensor.matmul(
                                    ps_s[:, 512 * jj:512 * jj + 512],
                                    kT[off:off + 64,
                                       128 * ki:128 * ki + 128],
                                    qTs[pp][off:off + 64,
                                            512 * qb:512 * qb + 512],
                                    start=True, stop=True)
                            box["ps"] = ps_s

                        def ec():
                            et = etp.tile([128, 1024], BF16, name="et",
                                          tag="et", bufs=8)
                            nc.scalar.activation(et, box["ps"], AF.Exp,
                                                 scale=SCALE)
                            for jj in range(2):
                                ki = kp + jj
                                ctx_mms(et[:, 512 * jj:512 * jj + 512],
                                        ki, [0, 1, 2, 3])
                        return (s, ec)

                    def mk_diag(which):
                        # which=0: j=0 (span 512 @0) + j=1 (span 384 @512)
                        # which=1: j=2 (span 256 @0) + j=3 (span 128 @256)
                        box = {}
                        js = (0, 1) if which == 0 else (2, 3)
                        offs = (0, 512) if which == 0 else (0, 256)

                        def s():
                            ps_s = psS.tile([128, 1024], F32, name="ps_dg",
                                            tag="ps_s", bufs=2)
                            for j, o in zip(js, offs):
                                ki = 4 * qb + j
                                span = 512 - 128 * j
                                nc.tensor.matmul(
                                    ps_s[:, o:o + span],
                                    kT[off:off + 64,
                                       128 * ki:128 * ki + 128],
                                    qTs[pp][off:off + 64,
                                            512 * qb + 128 * j:
                                            512 * (qb + 1)],
                                    start=True, stop=True)
                            box["ps"] = ps_s

                        def ec():
                            wid = 896 if which == 0 else 384
                            et = etp.tile([128, 1024], BF16, name="etd",
                                          tag="et", bufs=8)
                            nc.scalar.activation(et[:, 0:wid],
                                                 box["ps"][:, 0:wid],
                                                 AF.Exp, scale=SCALE)
                            # mask the diagonal 128x128 chunk of each j
                            for j, o in zip(js, offs):
                                eng = nc.vector
                                eng.tensor_mul(et[:, o:o + 128],
                                               et[:, o:o + 128], m01_sb)
                            for j, o in zip(js, offs):
                                ki = 4 * qb + j
                                span = et[:, o:o + 512 - 128 * j]
                                jl = list(range(j, 4))
                                # unmasked q-subtiles first; the masked
                                # diagonal chunk (qt==j) last
                                ctx_mms(span, ki, jl[1:], base_idx=1)
                                ctx_mms(span, ki, jl[:1], base_idx=0)
                        return (s, ec)

                    if causal:
                        units.append(mk_diag(0))
                        units.append(mk_diag(1))
                    for kp in range(0, nfull, 2):
                        units.append(mk_pair(kp))

                    def phase_c(ctxns_h):
                        rc = sbC.tile([128, 4, 1], F32, name="rc", tag="rc",
                                      bufs=4)
                        nc.vector.reciprocal(rc, ps_ctx[:, :, 64:65])
                        for j in range(4):
                            cn = sbC.tile([128, 64], BF16, name="ctxn",
                                          tag="ctxn", bufs=16)
                            nc.vector.tensor_scalar_mul(
                                cn, ps_ctx[:, j, 0:64], rc[:, j, 0:1])
                            ctxns_h[j] = cn

                    return units, phase_c

                def emit_bc(qb, sp, post_flush=None, queue_fillers=None,
                            late_fillers=None):
                    # attention for one (q block, head pair); the two heads'
                    # unit streams interleave so ACT always has a ready exp
                    ensure(f"q{sp}sc{qb}")
                    if qb > 0:
                        ensure(f"kvsc{qb}")
                    ctxns = [[None] * 4 for _ in range(2)]
                    u0, pc0 = build_head(qb, 0, sp)
                    u1, pc1 = build_head(qb, 1, sp)
                    n = len(u0)
                    u0[0][0]()
                    if post_flush is not None:
                        flush_pending()
                        post_flush()
                    if queue_fillers is not None:
                        queue_fillers()
                    u1[0][0]()
                    drain(1)
                    for i in range(n):
                        if i == 1:
                            # flush the previous block's transposes one
                            # round in, when their normalize chain is done
                            flush_pending()
                            if late_fillers is not None:
                                late_fillers()
                        if i + 1 < n:
                            u0[i + 1][0]()
                            drain(1)
                        u0[i][1]()
                        if i + 1 < n:
                            u1[i + 1][0]()
                            drain(1)
                        u1[i][1]()
                    pc0(ctxns[0])
                    pc1(ctxns[1])

                    def do_transposes(_sp=sp, _qb=qb, _ctxns=ctxns):
                        # 8 transposed chunks share one psum zero-region:
                        # first start=True zeroes it, the rest accumulate
                        # into disjoint (zeroed) addresses
                        ps_t = psD.tile([128, 512], F32, name="ps_t",
                                        tag="ps_d", bufs=2).bitcast(BF16)
                        for hh in range(2):
                            base = 64 * hh
                            for j in range(4):
                                nc.tensor.matmul(
                                    ps_t[base:base + 64,
                                         128 * j:128 * j + 128],
                                    _ctxns[hh][j], idb_sb[:, 0:128],
                                    start=(j == 0), stop=(j == 3),
                                    is_transpose=True)
                        for hh in range(2):
                            nc.vector.tensor_copy(
                                ctxTs[hh][_qb][64 * _sp:64 * _sp + 64, :],
                                ps_t[64 * hh:64 * hh + 64, 0:512])
                    pending_c.append(do_transposes)

                # ---- global emission order ----
                queue_kv_sc(0, direct=True)
                queue_q_sc(0, 0)
                ensure("q0sc0")
                queue_q_sc(1, 0)
                queue_kv_sc(1)
                queue_q_sc(0, 1)
                emit_bc(0, 0)
                queue_kv_sc(2)
                queue_q_sc(1, 1)
                emit_bc(0, 1)
                queue_q_sc(0, 2)
                queue_q_sc(1, 2)
                emit_bc(1, 0)
                queue_kv_sc(3)
                emit_bc(1, 1)

                def q20_fill():
                    queue_q_sc(0, 3)
                    queue_q_sc(1, 3)
                emit_bc(2, 0, queue_fillers=q20_fill)
                emit_bc(2, 1, queue_fillers=lambda: queue_phase_d(0))
                emit_bc(3, 0, queue_fillers=lambda: queue_phase_d(1),
                        late_fillers=lambda: queue_phase_d(2, [8, 9]))
                emit_bc(3, 1, queue_fillers=lambda: queue_phase_d(2, [10, 11]))
                drain_all()
                flush_pending()
                # tail: last q block's output projection, double-width psum
                # slots from the now-idle attention ring for deep pipelining
                for qt in range(12, 16):
                    ps_o = psS.tile([128, 1024], F32, name="ps_ow",
                                    tag="ps_s", bufs=2)
                    col = 128 * (qt % 4)
                    for nb in range(2):
                        for c in range(2):
                            nc.tensor.matmul(
                                ps_o[:, 512 * nb:512 * nb + 512],
                                ctxTs[c][3][:, col:col + 128],
                                wo_sb[:, c, 512 * nb:512 * nb + 512],
                                start=(c == 0), stop=(c == 1))
                    ost = sbC.tile([128, 1024], BF16, name="ostw",
                                   tag="ostw", bufs=4)
                    nc.scalar.copy(ost[:, 0:512], ps_o[:, 0:512])
                    nc.vector.tensor_copy(ost[:, 512:1024], ps_o[:, 512:1024])
                    nc.sync.dma_start(
                        out=outp[128 * qt:128 * qt + 128, :], in_=ost)

    nc.compile()
    return nc


_NC_CACHE = {}


def _get_nc(causal: bool):
    if causal not in _NC_CACHE:
        _NC_CACHE[causal] = _build_nc(causal)
    return _NC_CACHE[causal]


def _host_consts():
    p = np.zeros((128, 128), np.float32)
    idx = np.arange(0, 128, 2)
    p[idx, idx + 1] = -1.0
    p[idx + 1, idx] = 1.0
    psigT = np.ascontiguousarray(p.T)
    ident = np.eye(128, dtype=np.float32)
    ident[64:128, 0:64] = np.eye(64, dtype=np.float32)
    idb = np.eye(128, dtype=ml_dtypes.bfloat16)
    m01 = (np.arange(128)[None, :] >= np.arange(128)[:, None])
    m01 = m01.astype(ml_dtypes.bfloat16)
    return psigT, ident, idb, m01


def _numpy_reference(hidden_states, cos, sin, attention_mask, Wq, Wk, Wv, Wo):
    """Generic-mask fallback, pure numpy port of the reference."""
    GROUPS = H // KVH

    def rope(x, c, s):
        c = c[:, None, :, :]
        s = s[:, None, :, :]
        x1, x2 = x[..., ::2], x[..., 1::2]
        xr = np.stack([x1 * c - x2 * s, x1 * s + x2 * c], axis=-1)
        return xr.reshape(x.shape)

    b, sq, d = hidden_states.shape
    q = (hidden_states @ Wq).reshape(b, sq, H, HD).transpose(0, 2, 1, 3)
    k = (hidden_states @ Wk).reshape(b, sq, KVH, HD).transpose(0, 2, 1, 3)
    v = (hidden_states @ Wv).reshape(b, sq, KVH, HD).transpose(0, 2, 1, 3)
    q = rope(q, cos, sin)
    k = rope(k, cos, sin)
    k = np.repeat(k, GROUPS, axis=1)
    v = np.repeat(v, GROUPS, axis=1)
    out = np.zeros((b, sq, d), np.float32)
    for bi in range(b):
        for hi in range(H):
            sc = (q[bi, hi] @ k[bi, hi].T) * SCALE + attention_mask[0, 0]
            sc = sc - sc.max(axis=-1, keepdims=True)
            e = np.exp(sc)
            pr = e / e.sum(axis=-1, keepdims=True)
            ctx = pr @ v[bi, hi]
            out[bi] += ctx @ Wo[hi * HD:(hi + 1) * HD]
    return out


def _make_in_maps(hs, cos, sin, Wq, Wk, Wv, Wo):
    psigT, ident, idb, m01 = _host_consts()
    chan_half = (np.arange(64) // 2)

    in_maps = []
    for core in range(8):
        b, t = core // TP, core % TP
        hT = np.ascontiguousarray(hs[b].T).astype(ml_dtypes.bfloat16)
        cs64v = np.ascontiguousarray(cos[b].T[chan_half, :])
        sn64v = np.ascontiguousarray(sin[b].T[chan_half, :])
        cs128v = np.ascontiguousarray(np.concatenate([cs64v, cs64v], axis=0)).astype(ml_dtypes.bfloat16)
        sn128v = np.ascontiguousarray(np.concatenate([sn64v, sn64v], axis=0)).astype(ml_dtypes.bfloat16)
        wq_s = Wq[:, t * 256:(t + 1) * 256].reshape(8, 128, 256)
        wq_s = np.ascontiguousarray(
            wq_s.transpose(1, 0, 2)).astype(ml_dtypes.bfloat16)
        wkv_s = np.concatenate([Wk[:, t * 64:(t + 1) * 64],
                                Wv[:, t * 64:(t + 1) * 64]],
                               axis=1).reshape(8, 128, 128)
        wkv_s = np.ascontiguousarray(
            wkv_s.transpose(1, 0, 2)).astype(ml_dtypes.bfloat16)
        wo_s = Wo[t * 256:(t + 1) * 256]
        # ctxT channel order per chunk: c0 = [h0|h2], c1 = [h1|h3]
        wo_p = np.concatenate([wo_s[0:64], wo_s[128:192],
                               wo_s[64:128], wo_s[192:256]],
                              axis=0).reshape(2, 128, 1024)
        wo_p = np.ascontiguousarray(
            wo_p.transpose(1, 0, 2)).astype(ml_dtypes.bfloat16)
        in_maps.append({
            "hT": hT, "cs128": cs128v, "sn128": sn128v,
            "wq": wq_s, "wkv": wkv_s, "wo": wo_p,
            "psigT": psigT, "ident": ident, "idb": idb, "m01": m01,
        })
    return in_maps


def kernel(**inputs) -> np.ndarray:
    hs = np.asarray(inputs["hidden_states"], np.float32)
    cos = np.asarray(inputs["cos"], np.float32)
    sin = np.asarray(inputs["sin"], np.float32)
    mask = np.asarray(inputs["attention_mask"], np.float32)
    Wq = np.asarray(inputs["Wq"], np.float32)
    Wk = np.asarray(inputs["Wk"], np.float32)
    Wv = np.asarray(inputs["Wv"], np.float32)
    Wo = np.asarray(inputs["Wo"], np.float32)

    m = mask.reshape(S, S)
    tril = np.tril(np.ones((S, S), dtype=bool))
    causal_ref = np.where(tril, np.float32(0.0), np.float32(NEG))
    if np.array_equal(m, causal_ref):
        causal = True
    elif not m.any():
        causal = False
    else:
        return _numpy_reference(hs, cos, sin, mask, Wq, Wk, Wv, Wo)

    nc = _get_nc(causal)
    in_maps = _make_in_maps(hs, cos, sin, Wq, Wk, Wv, Wo)
    res = run_bass_kernel_spmd(nc, in_maps, core_ids=list(range(8)))
    out = np.zeros((B, S, D), np.float32)
    for core in range(8):
        out[core // TP] += res.results[core]["out"].astype(np.float32)
    return out


# revision 14
# speedup vs baseline: 1.0577x; 1.0224x over previous
"""Self-contained Trainium2 Bass kernel for GQA MultiHeadAttention with RoPE.

Problem: B=2, S=2048, D=1024, H=16 Q heads, KVH=4 KV heads, head_dim=64,
causal additive mask, f32.

Sharding: tensor-parallel over heads (TP=4: 4 Q heads + 1 KV head per shard)
x data-parallel over batch (DP=2) = 8 NeuronCores. Wo is sharded on its
input dim; the host sums the 4 partial outputs per batch element.

Design notes (tuned against the TimelineSim cost model, HW-validated):
- ctx matmuls use probs as the STATIONARY operand (out = [128 qpos, 65]
  per k-tile, Ldweights is free) cutting ctx PE cost ~2.3x vs V-stationary,
  and putting the softmax rowsum per-partition: normalization is a single
  DVE reciprocal + tensor_scalar, no cross-partition reductions.
- exp is the only ACT-engine work; all PSUM evictions ride ACT (early,
  while idle) or DVE (GPSIMD cannot touch PSUM on real HW).
- the two heads of each (q-block, head-pair) interleave unit-by-unit so
  ACT always has a ready exp; K/V+Q projections and the output projection
  are chopped into ~0.4us filler thunks drained one-per-attention-unit,
  queued as late as dependencies allow so the late ACT-bound blocks stay
  fed; diag (masked) units run first within each block.
- all four ctx accumulation chains of a head share one 2KB PSUM bank:
  first-emitted matmul start=True zeroes the zero-region, the rest
  accumulate into disjoint addresses, last-emitted carries stop=True.
- bf16 everywhere precision allows (q/k/probs/V/ctxT/Wo/cos/sin/output
  partials); fp32 PSUM accumulation throughout keeps rel err ~4e-3.
"""

import os
import sys

for _p in ("/opt/trn_rl_repo", "/root/.axon_site/_ro/trn_rl_repo"):
    if os.path.isdir(_p) and _p not in sys.path:
        sys.path.insert(0, _p)

import numpy as np
import ml_dtypes

import concourse.bacc as bacc
import concourse.bass as bass
import concourse.tile as tile
from concourse import mybir
from concourse.bass_utils import run_bass_kernel_spmd

F32 = mybir.dt.float32
F32R = mybir.dt.float32r
BF16 = mybir.dt.bfloat16
AF = mybir.ActivationFunctionType

H, KVH, HD = 16, 4, 64
B, S, D = 2, 2048, 1024
TP = 4                      # head-parallel ways
SCALE = HD ** -0.5
NEG = -1e9
NT = S // 128               # 16 kv tiles
NQB = S // 512              # 4 q blocks


def _patch_act_tables():
    """Make Exp resolve only to natural_log_exp_and_others so the
    act-table-load pass emits one load instead of thrashing."""
    from concourse.hw_specs import get_activation_tables
    t = get_activation_tables("gen3")
    for name, fns in t.items():
        if name != "natural_log_exp_and_others":
            fns.discard(AF.Exp)
            fns.discard(AF.Ln)


def _build_nc(causal: bool):
    _patch_act_tables()
    nc = bacc.Bacc()

    hT = nc.declare_dram_parameter("hT", [D, S], BF16, isOutput=False)
    cs128 = nc.declare_dram_parameter("cs128", [128, S], BF16, isOutput=False)
    sn128 = nc.declare_dram_parameter("sn128", [128, S], BF16, isOutput=False)
    wq = nc.declare_dram_parameter("wq", [128, 8, 256], BF16, isOutput=False)
    wkv = nc.declare_dram_parameter("wkv", [128, 8, 128], BF16, isOutput=False)
    wo = nc.declare_dram_parameter("wo", [128, 2, D], BF16, isOutput=False)
    psigT = nc.declare_dram_parameter("psigT", [128, 128], F32R, isOutput=False)
    ident = nc.declare_dram_parameter("ident", [128, 128], F32R, isOutput=False)
    idb = nc.declare_dram_parameter("idb", [128, 128], BF16, isOutput=False)
    m01 = nc.declare_dram_parameter("m01", [128, 128], BF16, isOutput=False)
    outp = nc.declare_dram_parameter("out", [S, D], BF16, isOutput=True)

    with tile.TileContext(nc) as tc:
        with tc.tile_pool(name="hold", bufs=1) as hp:
            # load order matters: K/V projection inputs first so compute can
            # start ASAP; second hidden half + Wo and small consts later
            wkv_sb = hp.tile([128, 8, 128], BF16, name="wkv_sb", tag="wkv_sb")
            nc.sync.dma_start(out=wkv_sb, in_=wkv[:, :, :])
            ht_sb = [hp.tile([128, S], BF16, name=f"ht{c}", tag=f"ht{c}")
                     for c in range(8)]
            for c in range(8):
                eng = nc.sync if c % 2 == 0 else nc.gpsimd
                eng.dma_start(out=ht_sb[c][:, 0:1024],
                              in_=hT[c * 128:(c + 1) * 128, 0:1024])
            psig_sb = hp.tile([128, 128], F32R, name="psig_sb", tag="psig_sb")
            nc.sync.dma_start(out=psig_sb, in_=psigT[:, :])
            cosf_sb = hp.tile([128, S], BF16, name="cosf_sb", tag="cosf_sb")
            sinf_sb = hp.tile([128, S], BF16, name="sinf_sb", tag="sinf_sb")
            nc.gpsimd.dma_start(out=cosf_sb[:, 0:1024], in_=cs128[:, 0:1024])
            nc.sync.dma_start(out=sinf_sb[:, 0:1024], in_=sn128[:, 0:1024])
            id_sb = hp.tile([128, 128], F32R, name="id_sb", tag="id_sb")
            nc.sync.dma_start(out=id_sb, in_=ident[:, :])
            wq_sb = hp.tile([128, 8, 256], BF16, name="wq_sb", tag="wq_sb")
            nc.sync.dma_start(out=wq_sb, in_=wq[:, :, :])
            nc.sync.dma_start(out=cosf_sb[:, 1024:2048], in_=cs128[:, 1024:2048])
            nc.sync.dma_start(out=sinf_sb[:, 1024:2048], in_=sn128[:, 1024:2048])
            m01_sb = hp.tile([128, 128], BF16, name="m01_sb", tag="m01_sb")
            nc.sync.dma_start(out=m01_sb, in_=m01[:, :])
            for c in range(8):
                nc.sync.dma_start(out=ht_sb[c][:, 1024:2048],
                                  in_=hT[c * 128:(c + 1) * 128, 1024:2048])
            idb_sb = hp.tile([128, 128], BF16, name="idb_sb", tag="idb_sb")
            nc.sync.dma_start(out=idb_sb, in_=idb[:, :])
            wo_sb = hp.tile([128, 2, D], BF16, name="wo_sb", tag="wo_sb")
            nc.sync.dma_start(out=wo_sb, in_=wo[:, :, :])

            qTs = [hp.tile([128, S], BF16, name=f"qT{p}", tag=f"qT{p}")
                   for p in range(2)]
            kT = hp.tile([128, S], BF16, name="kTt", tag="kTt")
            vsm = hp.tile([128, NT, 65], BF16, name="vsm", tag="vsm")
            ctxTs = [[hp.tile([128, 512], BF16, name=f"ctxT{c}_{q}",
                              tag=f"ctxT{c}_{q}") for q in range(NQB)]
                     for c in range(2)]

            # ones column (65th) of vsm for the softmax denominator
            nc.vector.memset(vsm[:, :, 64:65], 1.0)

            with tc.tile_pool(name="psS", bufs=1, space="PSUM") as psS, \
                 tc.tile_pool(name="psC", bufs=1, space="PSUM") as psC, \
                 tc.tile_pool(name="psD", bufs=1, space="PSUM") as psD, \
                 tc.tile_pool(name="etp", bufs=1) as etp, \
                 tc.tile_pool(name="sbA", bufs=4) as sbA, \
                 tc.tile_pool(name="sbC", bufs=1) as sbC:

                # ---- filler queue: small PE-work thunks drained one per
                # attention unit so projections/output ride the exp shadow
                filler = []

                def drain(n=1):
                    for _ in range(n):
                        if filler:
                            filler.pop(0)[1]()

                def ensure(label):
                    rest, todo = [], []
                    for it in filler:
                        (todo if it[0] == label else rest).append(it)
                    filler[:] = rest
                    for _, th in todo:
                        th()

                def drain_all():
                    while filler:
                        filler.pop(0)[1]()

                # deferred phase-C emission (transposes + ctxT evicts)
                pending_c = []

                def flush_pending():
                    while pending_c:
                        pending_c.pop(0)()

                # ---------------- Phase A: projections + rope ----------------
                def queue_q_sc(pp, sc):
                    label = f"q{pp}sc{sc}"
                    csl = slice(512 * sc, 512 * sc + 512)
                    box = {}

                    def proj_a():
                        ps_q = psD.tile([128, 512], F32, name="ps_q",
                                        tag="ps_d", bufs=2)
                        for dc in range(4):
                            nc.tensor.matmul(
                                ps_q,
                                wq_sb[:, dc, 128 * pp:128 * pp + 128],
                                ht_sb[dc][:, csl],
                                start=(dc == 0), stop=False)
                        box["ps"] = ps_q

                    def proj_b():
                        ps_q = box["ps"]
                        for dc in range(4, 8):
                            nc.tensor.matmul(
                                ps_q,
                                wq_sb[:, dc, 128 * pp:128 * pp + 128],
                                ht_sb[dc][:, csl],
                                start=False, stop=(dc == 7))

                    def rope():
                        ps_q = box["ps"]
                        qraw = sbA.tile([128, 512], F32R, name="qraw",
                                        tag="qraw")
                        nc.vector.tensor_copy(qraw, ps_q)
                        ps_rot = psD.tile([128, 512], F32, name="ps_rot",
                                          tag="ps_d", bufs=2)
                        nc.tensor.matmul(ps_rot, psig_sb.bitcast(F32R),
                                         qraw.bitcast(F32R),
                                         start=True, stop=True)
                        qc = sbA.tile([128, 512], F32, name="qc", tag="qc")
                        nc.gpsimd.tensor_mul(qc, qraw.bitcast(F32),
                                             cosf_sb[:, csl])
                        rtmp = sbA.tile([128, 512], F32, name="rtmp",
                                        tag="rtmp")
                        nc.vector.tensor_mul(rtmp, ps_rot, sinf_sb[:, csl])
                        nc.gpsimd.tensor_add(qTs[pp][:, csl], qc, rtmp)

                    filler.append((label, proj_a))
                    filler.append((label, proj_b))
                    filler.append((label, rope))

                def queue_kv_sc(sc, direct=False):
                    # K/V: kvT = [Wk|Wv].T @ h.T -> K rows 0:64, V rows 64:128
                    label = f"kvsc{sc}"
                    csl = slice(512 * sc, 512 * sc + 512)
                    box = {}

                    def proj_a():
                        ps_kv = psD.tile([128, 512], F32, name="ps_kv",
                                         tag="ps_d", bufs=2)
                        for dc in range(4):
                            nc.tensor.matmul(
                                ps_kv,
                                wkv_sb[:, dc, :],
                                ht_sb[dc][:, csl],
                                start=(dc == 0), stop=False)
                        box["ps"] = ps_kv

                    def proj_b():
                        ps_kv = box["ps"]
                        for dc in range(4, 8):
                            nc.tensor.matmul(
                                ps_kv,
                                wkv_sb[:, dc, :],
                                ht_sb[dc][:, csl],
                                start=False, stop=(dc == 7))

                    def krope():
                        kvraw = sbA.tile([128, 512], F32R, name="kvraw",
                                         tag="kvraw")
                        nc.vector.tensor_copy(kvraw, box["ps"])
                        box["kvraw"] = kvraw
                        # rope on K rows
                        ps_krot = psD.tile([128, 512], F32, name="ps_krot",
                                           tag="ps_d", bufs=2)[0:64, :]
                        nc.tensor.matmul(ps_krot,
                                         psig_sb[0:64, 0:64].bitcast(F32R),
                                         kvraw[0:64, :].bitcast(F32R),
                                         start=True, stop=True)
                        kc = sbA.tile([64, 512], F32, name="kc", tag="kc")
                        nc.gpsimd.tensor_mul(kc, kvraw[0:64, :].bitcast(F32),
                                             cosf_sb[0:64, csl])
                        ktmp = sbA.tile([64, 512], F32, name="ktmp", tag="ktmp")
                        nc.vector.tensor_mul(ktmp, ps_krot, sinf_sb[0:64, csl])
                        nc.gpsimd.tensor_add(kT[0:64, csl], kc, ktmp)
                        # duplicate roped K to partitions 64:128 (engines
                        # cannot cross partitions; DMA can)
                        nc.sync.dma_start(out=kT[64:128, csl],
                                          in_=kT[0:64, csl])

                    def vt(pair):
                        # V: transpose 128-seq tiles into vsm (seq-major)
                        kvraw = box["kvraw"]
                        for tt in (2 * pair, 2 * pair + 1):
                            ti = 4 * sc + tt
                            ps_v = psD.tile([128, 512], F32, name="ps_v",
                                            tag="ps_d", bufs=2)[:, 0:64]
                            nc.tensor.matmul(
                                ps_v.bitcast(F32R),
                                kvraw[64:128, 128 * tt:128 * tt + 128].bitcast(F32R),
                                id_sb[64:128, 0:64].bitcast(F32R),
                                start=True, stop=True, is_transpose=True)
                            nc.vector.tensor_copy(vsm[:, ti, 0:64], ps_v)

                    thunks = [proj_a, proj_b, krope,
                              lambda: vt(0), lambda: vt(1)]
                    if direct:
                        for th in thunks:
                            th()
                    else:
                        for th in thunks:
                            filler.append((label, th))

                def queue_phase_d(dq, qts=None):
                    label = f"pd{dq}"
                    for qt in (qts if qts is not None
                               else range(4 * dq, 4 * dq + 4)):
                        ct0 = ctxTs[0][qt // 4]
                        ct1 = ctxTs[1][qt // 4]
                        col = 128 * (qt % 4)
                        for nb in range(2):
                            def th(_qt=qt, _nb=nb, _ct0=ct0, _ct1=ct1,
                                   _col=col):
                                ps_o = psD.tile([128, 512], F32, name="ps_o",
                                                tag="ps_d", bufs=2)
                                nc.tensor.matmul(
                                    ps_o, _ct0[:, _col:_col + 128],
                                    wo_sb[:, 0, 512 * _nb:512 * _nb + 512],
                                    start=True, stop=False)
                                nc.tensor.matmul(
                                    ps_o, _ct1[:, _col:_col + 128],
                                    wo_sb[:, 1, 512 * _nb:512 * _nb + 512],
                                    start=False, stop=True)
                                ost = sbC.tile([128, 512], BF16, name="ost",
                                               tag="ost", bufs=6)
                                nc.vector.tensor_copy(ost, ps_o)
                                nc.sync.dma_start(
                                    out=outp[128 * _qt:128 * _qt + 128,
                                             512 * _nb:512 * _nb + 512],
                                    in_=ost)
                            filler.append((label, th))

                def build_head(qb, hh, sp):
                    h = 2 * sp + hh
                    off = 64 * (h % 2)
                    pp = h // 2
                    ps_ctx = psC.tile([128, 4, 65], F32, name=f"ps_ctx{hh}",
                                      tag="ps_ctx", bufs=2)
                    nfull = (4 * qb) if causal else NT
                    lastki = (4 * qb + 3) if causal else (NT - 1)
                    units = []

                    ctx_total = (16 * qb + 10) if causal else 64
                    ctx_cnt = [0]

                    def ctx_mms(et_ap, ki, jlist, base_idx=0):
                        # et_ap: probs chunk row; one [q,65] matmul per
                        # q-subtile j (chunk at base_idx+n within et_ap).
                        # All 4 q-subtile chains share one psum bank: the
                        # FIRST EMITTED matmul's start=True lazily zeroes
                        # the whole 2KB zero-region, every later matmul
                        # accumulates (disjoint addresses read as zero), and
                        # the LAST one closes the group with stop=True.
                        for idx, j in enumerate(jlist):
                            start = (ctx_cnt[0] == 0)
                            stop = (ctx_cnt[0] == ctx_total - 1)
                            ctx_cnt[0] += 1
                            o = 128 * (base_idx + idx)
                            nc.tensor.matmul(
                                ps_ctx[:, j, :],
                                et_ap[:, o:o + 128],
                                vsm[:, ki, 0:65],
                                start=start, stop=stop)

                    def mk_pair(kp):
                        box = {}

                        def s():
                            ps_s = psS.tile([128, 1024], F32, name="ps_s",
                                            tag="ps_s", bufs=2)
                            for jj in range(2):
                                ki = kp + jj
                                nc.tensor.matmul(
                                    ps_s[:, 512 * jj:512 * jj + 512],
                                    kT[off:off + 64,
                                       128 * ki:128 * ki + 128],
                                    qTs[pp][off:off + 64,
                                            512 * qb:512 * qb + 512],
                                    start=True, stop=True)
                            box["ps"] = ps_s

                        def ec():
                            et = etp.tile([128, 1024], BF16, name="et",
                                          tag="et", bufs=8)
                            nc.scalar.activation(et, box["ps"], AF.Exp,
                                                 scale=SCALE)
                            for jj in range(2):
                                ki = kp + jj
                                ctx_mms(et[:, 512 * jj:512 * jj + 512],
                                        ki, [0, 1, 2, 3])
                        return (s, ec)

                    def mk_diag(which):
                        # which=0: j=0 (span 512 @0) + j=1 (span 384 @512)
                        # which=1: j=2 (span 256 @0) + j=3 (span 128 @256)
                        box = {}
                        js = (0, 1) if which == 0 else (2, 3)
                        offs = (0, 512) if which == 0 else (0, 256)

                        def s():
                            ps_s = psS.tile([128, 1024], F32, name="ps_dg",
                                            tag="ps_s", bufs=2)
                            for j, o in zip(js, offs):
                                ki = 4 * qb + j
                                span = 512 - 128 * j
                                nc.tensor.matmul(
                                    ps_s[:, o:o + span],
                                    kT[off:off + 64,
                                       128 * ki:128 * ki + 128],
                                    qTs[pp][off:off + 64,
                                            512 * qb + 128 * j:
                                            512 * (qb + 1)],
                                    start=True, stop=True)
                            box["ps"] = ps_s

                        def ec():
                            wid = 896 if which == 0 else 384
                            et = etp.tile([128, 1024], BF16, name="etd",
                                          tag="et", bufs=8)
                            nc.scalar.activation(et[:, 0:wid],
                                                 box["ps"][:, 0:wid],
                                                 AF.Exp, scale=SCALE)
                            # mask the diagonal 128x128 chunk of each j
                            for j, o in zip(js, offs):
                                eng = nc.vector
                                eng.tensor_mul(et[:, o:o + 128],
                                               et[:, o:o + 128], m01_sb)
                            for j, o in zip(js, offs):
                                ki = 4 * qb + j
                                span = et[:, o:o + 512 - 128 * j]
                                jl = list(range(j, 4))
                                # unmasked q-subtiles first; the masked
                                # diagonal chunk (qt==j) last
                                ctx_mms(span, ki, jl[1:], base_idx=1)
                                ctx_mms(span, ki, jl[:1], base_idx=0)
                        return (s, ec)

                    if causal:
                        units.append(mk_diag(0))
                        units.append(mk_diag(1))
                    for kp in range(0, nfull, 2):
                        units.append(mk_pair(kp))

                    def phase_c(ctxns_h):
                        rc = sbC.tile([128, 4, 1], F32, name="rc", tag="rc",
                                      bufs=4)
                        nc.vector.reciprocal(rc, ps_ctx[:, :, 64:65])
                        for j in range(4):
                            cn = sbC.tile([128, 64], BF16, name="ctxn",
                                          tag="ctxn", bufs=16)
                            nc.vector.tensor_scalar_mul(
                                cn, ps_ctx[:, j, 0:64], rc[:, j, 0:1])
                            ctxns_h[j] = cn

                    return units, phase_c

                def emit_bc(qb, sp, post_flush=None, queue_fillers=None,
                            late_fillers=None):
                    # attention for one (q block, head pair); the two heads'
                    # unit streams interleave so ACT always has a ready exp
                    ensure(f"q{sp}sc{qb}")
                    if qb > 0:
                        ensure(f"kvsc{qb}")
                    ctxns = [[None] * 4 for _ in range(2)]
                    u0, pc0 = build_head(qb, 0, sp)
                    u1, pc1 = build_head(qb, 1, sp)
                    n = len(u0)
                    # qb0 blocks are short and PE-lean: drain fillers at
                    # double rate so the early projection/rope chains finish
                    # long before their consumers
                    D = 2 if qb == 0 else 1
                    u0[0][0]()
                    if post_flush is not None:
                        flush_pending()
                        post_flush()
                    if queue_fillers is not None:
                        queue_fillers()
                    u1[0][0]()
                    drain(D)
                    for i in range(n):
                        if i == 1:
                            # flush the previous block's transposes one
                            # round in, when their normalize chain is done
                            flush_pending()
                            if late_fillers is not None:
                                late_fillers()
                        if i + 1 < n:
                            u0[i + 1][0]()
                            drain(D)
                        u0[i][1]()
                        if i + 1 < n:
                            u1[i + 1][0]()
                            drain(D)
                        u1[i][1]()
                    pc0(ctxns[0])
                    pc1(ctxns[1])

                    def do_transposes(_sp=sp, _qb=qb, _ctxns=ctxns):
                        # 8 transposed chunks share one psum zero-region:
                        # first start=True zeroes it, the rest accumulate
                        # into disjoint (zeroed) addresses
                        ps_t = psD.tile([128, 512], F32, name="ps_t",
                                        tag="ps_d", bufs=2).bitcast(BF16)
                        for hh in range(2):
                            base = 64 * hh
                            for j in range(4):
                                nc.tensor.matmul(
                                    ps_t[base:base + 64,
                                         128 * j:128 * j + 128],
                                    _ctxns[hh][j], idb_sb[:, 0:128],
                                    start=(j == 0), stop=(j == 3),
                                    is_transpose=True)
                        for hh in range(2):
                            nc.vector.tensor_copy(
                                ctxTs[hh][_qb][64 * _sp:64 * _sp + 64, :],
                                ps_t[64 * hh:64 * hh + 64, 0:512])
                    pending_c.append(do_transposes)

                # ---- global emission order ----
                queue_kv_sc(0, direct=True)
                queue_q_sc(0, 0)
                ensure("q0sc0")
                queue_q_sc(1, 0)
                queue_kv_sc(1)
                queue_q_sc(0, 1)
                emit_bc(0, 0)
                queue_kv_sc(2)
                queue_q_sc(1, 1)
                emit_bc(0, 1)
                queue_q_sc(0, 2)
                queue_q_sc(1, 2)
                emit_bc(1, 0)
                queue_kv_sc(3)
                emit_bc(1, 1)

                def q20_fill():
                    queue_q_sc(0, 3)
                    queue_q_sc(1, 3)
                emit_bc(2, 0, queue_fillers=q20_fill)
                emit_bc(2, 1, queue_fillers=lambda: queue_phase_d(0))
                emit_bc(3, 0, queue_fillers=lambda: queue_phase_d(1),
                        late_fillers=lambda: queue_phase_d(2, [8, 9]))
                emit_bc(3, 1, queue_fillers=lambda: queue_phase_d(2, [10, 11]))
                drain_all()
                flush_pending()
                # tail: last q block's output projection, double-width psum
                # slots from the now-idle attention ring for deep pipelining
                for qt in range(12, 16):
                    ps_o = psS.tile([128, 1024], F32, name="ps_ow",
                                    tag="ps_s", bufs=2)
                    col = 128 * (qt % 4)
                    for nb in range(2):
                        for c in range(2):
                            nc.tensor.matmul(
                                ps_o[:, 512 * nb:512 * nb + 512],
                                ctxTs[c][3][:, col:col + 128],
                                wo_sb[:, c, 512 * nb:512 * nb + 512],
                                start=(c == 0), stop=(c == 1))
                    ost = sbC.tile([128, 1024], BF16, name="ostw",
                                   tag="ostw", bufs=4)
                    nc.scalar.copy(ost[:, 0:512], ps_o[:, 0:512])
                    nc.vector.tensor_copy(ost[:, 512:1024], ps_o[:, 512:1024])
                    nc.sync.dma_start(
                        out=outp[128 * qt:128 * qt + 128, :], in_=ost)

    nc.compile()
    return nc


_NC_CACHE = {}


def _get_nc(causal: bool):
    if causal not in _NC_CACHE:
        _NC_CACHE[causal] = _build_nc(causal)
    return _NC_CACHE[causal]


def _host_consts():
    p = np.zeros((128, 128), np.float32)
    idx = np.arange(0, 128, 2)
    p[idx, idx + 1] = -1.0
    p[idx + 1, idx] = 1.0
    psigT = np.ascontiguousarray(p.T)
    ident = np.eye(128, dtype=np.float32)
    ident[64:128, 0:64] = np.eye(64, dtype=np.float32)
    idb = np.eye(128, dtype=ml_dtypes.bfloat16)
    m01 = (np.arange(128)[None, :] >= np.arange(128)[:, None])
    m01 = m01.astype(ml_dtypes.bfloat16)
    return psigT, ident, idb, m01


def _numpy_reference(hidden_states, cos, sin, attention_mask, Wq, Wk, Wv, Wo):
    """Generic-mask fallback, pure numpy port of the reference."""
    GROUPS = H // KVH

    def rope(x, c, s):
        c = c[:, None, :, :]
        s = s[:, None, :, :]
        x1, x2 = x[..., ::2], x[..., 1::2]
        xr = np.stack([x1 * c - x2 * s, x1 * s + x2 * c], axis=-1)
        return xr.reshape(x.shape)

    b, sq, d = hidden_states.shape
    q = (hidden_states @ Wq).reshape(b, sq, H, HD).transpose(0, 2, 1, 3)
    k = (hidden_states @ Wk).reshape(b, sq, KVH, HD).transpose(0, 2, 1, 3)
    v = (hidden_states @ Wv).reshape(b, sq, KVH, HD).transpose(0, 2, 1, 3)
    q = rope(q, cos, sin)
    k = rope(k, cos, sin)
    k = np.repeat(k, GROUPS, axis=1)
    v = np.repeat(v, GROUPS, axis=1)
    out = np.zeros((b, sq, d), np.float32)
    for bi in range(b):
        for hi in range(H):
            sc = (q[bi, hi] @ k[bi, hi].T) * SCALE + attention_mask[0, 0]
            sc = sc - sc.max(axis=-1, keepdims=True)
            e = np.exp(sc)
            pr = e / e.sum(axis=-1, keepdims=True)
            ctx = pr @ v[bi, hi]
            out[bi] += ctx @ Wo[hi * HD:(hi + 1) * HD]
    return out


def _make_in_maps(hs, cos, sin, Wq, Wk, Wv, Wo):
    psigT, ident, idb, m01 = _host_consts()
    chan_half = (np.arange(64) // 2)

    in_maps = []
    for core in range(8):
        b, t = core // TP, core % TP
        hT = np.ascontiguousarray(hs[b].T).astype(ml_dtypes.bfloat16)
        cs64v = np.ascontiguousarray(cos[b].T[chan_half, :])
        sn64v = np.ascontiguousarray(sin[b].T[chan_half, :])
        cs128v = np.ascontiguousarray(np.concatenate([cs64v, cs64v], axis=0)).astype(ml_dtypes.bfloat16)
        sn128v = np.ascontiguousarray(np.concatenate([sn64v, sn64v], axis=0)).astype(ml_dtypes.bfloat16)
        wq_s = Wq[:, t * 256:(t + 1) * 256].reshape(8, 128, 256)
        wq_s = np.ascontiguousarray(
            wq_s.transpose(1, 0, 2)).astype(ml_dtypes.bfloat16)
        wkv_s = np.concatenate([Wk[:, t * 64:(t + 1) * 64],
                                Wv[:, t * 64:(t + 1) * 64]],
                               axis=1).reshape(8, 128, 128)
        wkv_s = np.ascontiguousarray(
            wkv_s.transpose(1, 0, 2)).astype(ml_dtypes.bfloat16)
        wo_s = Wo[t * 256:(t + 1) * 256]
        # ctxT channel order per chunk: c0 = [h0|h2], c1 = [h1|h3]
        wo_p = np.concatenate([wo_s[0:64], wo_s[128:192],
                               wo_s[64:128], wo_s[192:256]],
                              axis=0).reshape(2, 128, 1024)
        wo_p = np.ascontiguousarray(
            wo_p.transpose(1, 0, 2)).astype(ml_dtypes.bfloat16)
        in_maps.append({
            "hT": hT, "cs128": cs128v, "sn128": sn128v,
            "wq": wq_s, "wkv": wkv_s, "wo": wo_p,
            "psigT": psigT, "ident": ident, "idb": idb, "m01": m01,
        })
    return in_maps


def kernel(**inputs) -> np.ndarray:
    hs = np.asarray(inputs["hidden_states"], np.float32)
    cos = np.asarray(inputs["cos"], np.float32)
    sin = np.asarray(inputs["sin"], np.float32)
    mask = np.asarray(inputs["attention_mask"], np.float32)
    Wq = np.asarray(inputs["Wq"], np.float32)
    Wk = np.asarray(inputs["Wk"], np.float32)
    Wv = np.asarray(inputs["Wv"], np.float32)
    Wo = np.asarray(inputs["Wo"], np.float32)

    m = mask.reshape(S, S)
    tril = np.tril(np.ones((S, S), dtype=bool))
    causal_ref = np.where(tril, np.float32(0.0), np.float32(NEG))
    if np.array_equal(m, causal_ref):
        causal = True
    elif not m.any():
        causal = False
    else:
        return _numpy_reference(hs, cos, sin, mask, Wq, Wk, Wv, Wo)

    nc = _get_nc(causal)
    in_maps = _make_in_maps(hs, cos, sin, Wq, Wk, Wv, Wo)
    res = run_bass_kernel_spmd(nc, in_maps, core_ids=list(range(8)))
    out = np.zeros((B, S, D), np.float32)
    for core in range(8):
        out[core // TP] += res.results[core]["out"].astype(np.float32)
    return out
